# revision 28
# baseline (speedup 1.0000x reference)
"""Trainium2 Bass kernel for nn_EnergyMovers (batched Sinkhorn OT loss).

Strategy (pure data parallelism, 4 batch elems per core x 8 cores):
  - Host: build masked augmented point vectors so d2[n,m] = sum_k A[k,n]*B[k,m]
    comes out of a K=4 TensorE matmul already masked (masked rows/cols -> d2=0
    -> K=exp(-sqrt(1e-12)/eps) ~ 1, matching the reference's logK=0 there).
  - Device per elem (A-layout only): d2 -> clamp(DVE) -> sqrt(ACT, fused
    [128,4*512] tile) -> exp(ACT, fused) -> raw K. Then:
      K'A = aw_n * K        via ACT Copy-with-per-partition-scale, whose
                            accum_out also yields rowsum(K'A) for the first
                            u-update (v0 = 1 incl. the reference's masked
                            exp(0)=1 columns) for free,
      K'B = bw_m * K^T      via 128x128 PE transposes of raw K + a
                            per-partition-scale DVE evacuation multiply,
      DK' = clamp(d2)*K'A   as 16 small DVE muls sprinkled one-per-phase
                            into the loop's DVE slack (only needed at the
                            final reduction).
  - Non-log Sinkhorn on reciprocal potentials (u = aw*U~, v = bw*V~):
        s_v = K'A^T @ U~ ; V~ = 1/s_v ; s_u = K'B^T @ V~ ; U~ = 1/s_u
    identical to the reference's log-domain iteration (f32/bf16 exponent
    range suffices).
  - Each phase is QUARTER-PIPELINED on the PE: the matvec output is split
    into four [128,128] PSUM quarters; each quarter's 4 contraction bursts
    (4 batch elems column-tiled at tile_position (0,32e), streaming
    concurrently) complete early, so its evacuation (alternating DVE/ACT,
    distinct tiles to avoid Tile's same-PSUM-reader serialization) and its
    selector matmul overlap the later quarters' streams. sel[32e,e]=1
    picks the 4 result rows -> pt16[128,(c,e)] partition-major, then ONE
    strided DVE reciprocal produces the next stationaries. The M=32
    stationary has the potential chunk in col 0, zeros elsewhere
    (zero-fills unused PSUM rows -> evacuations read no garbage).
  - A ~7us dummy-matmul burst right before the loop flips the HAM clock
    gate to 8/8 so the loop's matmuls run at 2.4 GHz; in-loop PE gaps stay
    well under the ~3.4us MID window so it never re-throttles.
  - Final: ot[e] = (DK'^T U~) . (bw * V~) via one more quarter-pipelined
    matvec + selector matmuls + DVE muls + ones-matmul partition reduction.
  - Host: huber(e) added, results gathered from 8 cores.
"""

import os
from contextlib import ExitStack

import numpy as np

import concourse.bass as bass
import concourse.bacc as bacc
import concourse.mybir as mybir
import concourse.tile as tile
from concourse.bass_utils import run_bass_kernel_spmd

N_CORES = 8
ELEMS = 4  # batch elements per core (B=32 / 8)
B, N, M = 32, 512, 512
EPS = 0.05
ITERS = int(os.environ.get("EM_ITERS", "50"))
F32 = mybir.dt.float32
BF16 = mybir.dt.bfloat16
AF = mybir.ActivationFunctionType
ALU = mybir.AluOpType


def _build_nc():
    nc = bacc.Bacc()
    ABaug = nc.declare_dram_parameter("ABaug", [ELEMS, 4, 2 * N],
                                      mybir.dt.float32r, isOutput=False)
    # wts cols: 0:16 aw[(c,e)], 16:32 bw[(c,e)]
    wtsp = nc.declare_dram_parameter("wts", [128, 32], F32, isOutput=False)
    eyep = nc.declare_dram_parameter("eye", [128, 128], F32, isOutput=False)
    otp = nc.declare_dram_parameter("ot", [1, ELEMS], F32, isOutput=True)
    # keeps the HAM warm-up matmuls live through dead-code elim
    scrp = nc.declare_dram_parameter("scr", [1, 1], F32, isOutput=True)

    with ExitStack() as ctx:
        tc = ctx.enter_context(tile.TileContext(nc))
        kpool = ctx.enter_context(tc.tile_pool(name="kmat", bufs=1))
        vpool = ctx.enter_context(tc.tile_pool(name="vec", bufs=1))

        # --- params / constants -----------------------------------------
        wt_sb = vpool.tile([128, 32], F32, tag="wt", name="wt")
        nc.sync.dma_start(out=wt_sb[:], in_=wtsp[:])
        aw_v = wt_sb[:, 0:16].rearrange("p (c e) -> p c e", c=4)
        bw_v = wt_sb[:, 16:32].rearrange("p (c e) -> p c e", c=4)
        eye_sb = vpool.tile([128, 128], F32, tag="eyef", name="eyef")
        nc.sync.dma_start(out=eye_sb[:], in_=eyep[:])
        identB = vpool.tile([128, 128], BF16, tag="identB", name="identB")
        nc.vector.tensor_copy(identB[:], eye_sb[:])
        ones = vpool.tile([128, 1], F32, tag="ones", name="ones")
        nc.gpsimd.memset(ones[:], 1.0)
        sel = vpool.tile([128, 4], BF16, tag="sel", name="sel")
        nc.gpsimd.memset(sel[:], 0.0)
        for e in range(ELEMS):
            nc.gpsimd.memset(sel[32 * e:32 * e + 1, e:e + 1], 1.0)
        outsb = vpool.tile([1, ELEMS], F32, tag="outsb", name="outsb")
        bias12 = vpool.tile([128, 1], F32, tag="bias12", name="bias12")
        nc.gpsimd.memset(bias12[:], 1e-12)

        # potentials: [128, (c, e, 32)] bf16; col 0 of each 32-block is the
        # live value, cols 1-31 stay zero (zero-pads the M=32 stationary).
        U_all = vpool.tile([128, 4, 4, 32], BF16, tag="U", name="U")
        V_all = vpool.tile([128, 4, 4, 32], BF16, tag="V", name="V")
        nc.gpsimd.memset(U_all[:], 0.0)
        nc.gpsimd.memset(V_all[:], 0.0)

        KA, KB, DK, CLA, AB_SB = {}, {}, {}, {}, {}
        for e in range(ELEMS):
            ab_sb = vpool.tile([4, 2 * N], mybir.dt.float32r,
                               tag=f"ABs{e}", name=f"ABs{e}")
            nc.sync.dma_start(out=ab_sb[:], in_=ABaug[e])
            AB_SB[e] = (ab_sb[:, 0:N], ab_sb[:, N:2 * N])
            KA[e] = kpool.tile([128, 4, 512], BF16, tag=f"KA{e}", name=f"KA{e}")
            KB[e] = kpool.tile([128, 4, 512], BF16, tag=f"KB{e}", name=f"KB{e}")
            DK[e] = kpool.tile([128, 4, 512], BF16, tag=f"DK{e}", name=f"DK{e}")
            CLA[e] = kpool.tile([128, 4, 512], F32, tag=f"cl{e}", name=f"cl{e}")

        rs = vpool.tile([128, 16], F32, tag="rs", name="rs")
        last_evac = {}
        with tc.tile_pool(name="pd2", bufs=2, space="PSUM") as pd2, \
             tc.tile_pool(name="ptp", bufs=2, space="PSUM") as ptp, \
             tc.tile_pool(name="kr", bufs=1) as krpool, \
             tc.tile_pool(name="st", bufs=4) as stpool:
            KRAW = {}
            last_sqrt = None
            # d2 (A layout) -> clamp -> sqrt, fused [128,2,512]/[128,4,512]
            for e in range(ELEMS):
                a_sb, b_sb = AB_SB[e]
                for h in range(2):
                    d2 = pd2.tile([128, 2, 512], F32, tag="d2", name="d2")
                    for c2 in range(2):
                        c = 2 * h + c2
                        nc.tensor.matmul(
                            d2[:, c2, :], a_sb[:, c * 128:(c + 1) * 128],
                            b_sb[:], start=True, stop=True,
                        )
                    nc.vector.tensor_scalar_max(
                        CLA[e][:, 2 * h:2 * h + 2, :], d2[:], 0.0
                    )
                st = stpool.tile([128, 4, 512], F32, tag="st", name="st")
                last_sqrt = nc.scalar.activation(
                    st[:], CLA[e][:], AF.Sqrt, bias=bias12[:]
                )
                KRAW[e] = (st, krpool.tile([128, 4, 512], BF16,
                                           tag=f"kr{e}", name=f"kr{e}"))
            # all exps after all sqrts (ACT table sets differ)
            exp_done = {}
            for e in range(ELEMS):
                st, kraw = KRAW[e]
                exp_inst = nc.scalar.activation(
                    kraw[:], st[:], AF.Exp, scale=-1.0 / EPS,
                )
                tile.add_dep_helper(exp_inst.ins, last_sqrt.ins,
                                    sync=True, reason="act-table-batch")
                exp_done[e] = exp_inst
            # K'A = aw * K via ACT Copy-with-scale; accum -> rowsums
            for e in range(ELEMS):
                kraw = KRAW[e][1]
                for c in range(4):
                    nc.scalar.activation(
                        KA[e][:, c, :], kraw[:, c, :], AF.Copy,
                        scale=aw_v[:, c, e:e + 1],
                        accum_out=rs[:, 4 * c + e:4 * c + e + 1],
                    )
            # K'B = bw * K^T via PE transposes + per-partition-scale evac
            for e in range(ELEMS):
                kraw = KRAW[e][1]
                for cm in range(4):
                    tp = ptp.tile([128, 4, 128], BF16, tag="tp", name="tp")
                    for ci in range(4):
                        nc.tensor.transpose(
                            tp[:, ci, :],
                            kraw[:, ci, cm * 128:(cm + 1) * 128], identB[:],
                        )
                    last_evac[(e, cm)] = nc.vector.tensor_scalar_mul(
                        KB[e][:, cm, :],
                        tp[:].rearrange("p a b -> p (a b)"),
                        bw_v[:, cm, e:e + 1],
                    )
            # first u-update: U~1 = aw / rowsum(K'A)  (v0 = 1 incl. masked)
            rs2 = vpool.tile([128, 16], F32, tag="rs2", name="rs2")
            nc.vector.tensor_scalar_max(rs2[:], rs[:], 1e-30)
            rcp = vpool.tile([128, 16], F32, tag="rcp", name="rcp")
            nc.vector.reciprocal(rcp[:], rs2[:])
            nc.vector.tensor_mul(
                U_all[:, :, :, 0],
                rcp[:].rearrange("p (c e) -> p c e", c=4), aw_v,
            )

        # --- Sinkhorn iterations ----------------------------------------
        with tc.tile_pool(name="psq", bufs=1, space="PSUM") as qpool, \
             tc.tile_pool(name="pt16", bufs=2, space="PSUM") as ptpool, \
             tc.tile_pool(name="dps", bufs=1, space="PSUM") as dpool, \
             tc.tile_pool(name="sb", bufs=2) as sbpool:

            dummy_ps = dpool.tile([1, 512], F32, tag="dps", name="dps")
            # warm-up burst: ~7us of back-to-back matmuls flips the HAM
            # clock gate to 8/8 right before the loop (gated so the
            # PE-idle ACT stretch of setup comes first)
            for i in range(16):
                wm = nc.tensor.matmul(
                    dummy_ps[:], sel[:, 0:1], KA[0][:, 0, :],
                    start=True, stop=True, skip_group_check=True,
                )
                if i == 0:
                    tile.add_dep_helper(wm.ins, last_evac[(1, 3)].ins,
                                        sync=True, reason="warmup-after-setup")

            PSQ = [qpool.tile([128, 256], F32, tag=f"q{q}", name=f"q{q}")
                   for q in range(2)]

            def emit_head(Kt, stat):
                # first two h0 chunk-bursts of a phase; only need the c0/c1
                # stationaries (the early reciprocal half, via subtile deps)
                for c in (0, 1):
                    for e in range(ELEMS):
                        nc.tensor.matmul(
                            PSQ[0][32 * e:32 * e + 32, :],
                            stat[:, c, e, :], Kt[e][:, c, 0:256],
                            start=(c == 0), stop=False,
                            tile_position=(0, 32 * e),
                        )

            def filler(n, src):
                # HAM-warmth fillers anchored on this phase's evacuated
                # tile so the scheduler cannot hoist them elsewhere
                for _ in range(n):
                    nc.tensor.matmul(
                        dummy_ps[:, 0:128], sel[:, 0:1], src[:, 0:128],
                        start=True, stop=True, skip_group_check=True,
                    )

            def phase(Kt, stat, out_all, nxt, head_done):
                """out_all[:,c,e,0] = 1 / (sum_c stat[:,c,e,:]^T @ Kt).
                Software-pipelined: the h0 c0/c1 bursts may have been
                pre-emitted into the previous phase's tail; this phase
                pre-emits nxt's in turn. Both evacuations run on ACT so
                the DVE only carries the critical-path reciprocals."""
                if not head_done:
                    emit_head(Kt, stat)
                for c in (2, 3):
                    for e in range(ELEMS):
                        nc.tensor.matmul(
                            PSQ[0][32 * e:32 * e + 32, :],
                            stat[:, c, e, :], Kt[e][:, c, 0:256],
                            start=False, stop=(c == 3),
                            tile_position=(0, 32 * e),
                        )
                sbq0 = sbpool.tile([128, 256], BF16, tag="sbq0", name="sbq0")
                nc.scalar.copy(sbq0[:], PSQ[0][:])
                for c in range(4):
                    for e in range(ELEMS):
                        nc.tensor.matmul(
                            PSQ[1][32 * e:32 * e + 32, :],
                            stat[:, c, e, :], Kt[e][:, c, 256:512],
                            start=(c == 0), stop=(c == 3),
                            tile_position=(0, 32 * e),
                        )
                sbq1 = sbpool.tile([128, 256], BF16, tag="sbq1", name="sbq1")
                nc.scalar.copy(sbq1[:], PSQ[1][:])
                SBQ = [sbq0, sbq1]
                pt16 = ptpool.tile([128, 4, 4], F32, tag="pt16", name="pt16")
                with nc.allow_low_precision("bf16 Sinkhorn potentials"):
                    for c in (0, 1):
                        nc.tensor.matmul(
                            pt16[:, c, :], sbq0[:, c * 128:(c + 1) * 128],
                            sel[:], start=True, stop=True,
                        )
                    filler(1, sbq0)
                    nc.vector.reciprocal(out_all[:, 0:2, :, 0],
                                         pt16[:, 0:2, :])
                    if nxt is not None:
                        emit_head(nxt[0], nxt[1])
                    for c in (2, 3):
                        nc.tensor.matmul(
                            pt16[:, c, :],
                            sbq1[:, (c - 2) * 128:(c - 1) * 128],
                            sel[:], start=True, stop=True,
                        )
                    filler(2, sbq1)
                    nc.vector.reciprocal(out_all[:, 2:4, :, 0],
                                         pt16[:, 2:4, :])
                return nxt is not None

            # reference order: 50x(u-update; v-update). u#1 done at setup.
            # DK' = clamp(d2)*K'A muls hide in the loop's DVE slack.
            dk_jobs = [(e, c) for e in range(ELEMS) for c in range(4)]
            jobs = []
            for _ in range(ITERS - 1):
                jobs.append((KA, U_all, V_all))
                jobs.append((KB, V_all, U_all))
            jobs.append((KA, U_all, V_all))
            head_done = False
            nphase = 0
            for i, (Kt, stat, out_all) in enumerate(jobs[:-1]):
                nxt = jobs[i + 1]
                head_done = phase(Kt, stat, out_all, nxt, head_done)
                nphase += 1
                if nphase - 1 < len(dk_jobs):
                    e, c = dk_jobs[nphase - 1]
                    nc.vector.tensor_mul(
                        DK[e][:, c, :], CLA[e][:, c, :], KA[e][:, c, :]
                    )
            for e, c in dk_jobs[nphase:]:  # leftovers (small ITERS only)
                nc.vector.tensor_mul(
                    DK[e][:, c, :], CLA[e][:, c, :], KA[e][:, c, :]
                )
            Kt, stat, out_all = jobs[-1]
            phase(Kt, stat, out_all, None, head_done)  # final v-update

            # --- final: ot[e] = (DK'^T U~) . (bw * V~) ------------------
            SBG = []
            for h in range(2):
                for c in range(4):
                    for e in range(ELEMS):
                        nc.tensor.matmul(
                            PSQ[h][32 * e:32 * e + 32, :],
                            U_all[:, c, e, :],
                            DK[e][:, c, 256 * h:256 * h + 256],
                            start=(c == 0), stop=(c == 3),
                            tile_position=(0, 32 * e),
                        )
                sbg = sbpool.tile([128, 256], BF16, tag=f"sbq{h}",
                                  name=f"sbg{h}")
                if h == 0:
                    nc.vector.tensor_copy(sbg[:], PSQ[h][:])
                else:
                    nc.scalar.copy(sbg[:], PSQ[h][:])
                SBG.append(sbg)
            ptg = ptpool.tile([128, 4, 4], F32, tag="pt16", name="ptg")
            for c in range(4):
                nc.tensor.matmul(
                    ptg[:, c, :],
                    SBG[c // 2][:, (c % 2) * 128:(c % 2 + 1) * 128],
                    sel[:], start=True, stop=True,
                )
            t1 = sbpool.tile([128, 4, 4], F32, tag="t1", name="t1")
            nc.vector.tensor_mul(t1[:], ptg[:], V_all[:, :, :, 0])
            t2 = sbpool.tile([128, 4, 4], F32, tag="t2", name="t2")
            nc.vector.tensor_mul(t2[:], t1[:], bw_v)
            r_e = sbpool.tile([128, 4], F32, tag="re", name="re")
            nc.vector.reduce_sum(
                r_e[:], t2[:].rearrange("p c e -> p e c"),
                axis=mybir.AxisListType.X,
            )
            po = ptpool.tile([1, 4], F32, tag="pt16", name="po")
            nc.tensor.matmul(po[:], ones[:], r_e[:], start=True, stop=True)
            nc.scalar.copy(outsb[:], po[:])
            nc.sync.dma_start(out=otp[:], in_=outsb[:])
            # keep the warm-up matmuls live
            scr_sb = vpool.tile([1, 1], F32, tag="scr", name="scr")
            nc.scalar.copy(scr_sb[:], dummy_ps[0:1, 0:1])
            nc.sync.dma_start(out=scrp[:], in_=scr_sb[:])
    nc.compile()
    return nc


_NC_CACHE = {}


def _get_nc():
    if "nc" not in _NC_CACHE:
        _NC_CACHE["nc"] = _build_nc()
    return _NC_CACHE["nc"]


def _host_prep(a_mask, pc_a, b_mask, pc_b):
    """Per-batch-element f32 prep mirroring the reference's masking."""
    f32 = np.float32
    a_pt = (a_mask * pc_a[..., 2]).astype(f32)          # [B,N]
    b_pt = (b_mask * pc_b[..., 2]).astype(f32)          # [B,M]
    va = (a_pt > 0).astype(f32)
    vb = (b_pt > 0).astype(f32)
    aw = (a_pt / a_pt.sum(axis=1, keepdims=True, dtype=f32)).astype(f32)
    bw = (b_pt / b_pt.sum(axis=1, keepdims=True, dtype=f32)).astype(f32)
    xa = pc_a[..., :2].astype(f32)                      # [B,N,2]
    xb = pc_b[..., :2].astype(f32)
    onesN = np.ones((B, N), f32)
    A = np.stack(
        [-2 * xa[..., 0], -2 * xa[..., 1],
         (xa * xa).sum(-1).astype(f32), onesN], axis=1
    ) * va[:, None, :]                                  # [B,4,N]
    Bm = np.stack(
        [xb[..., 0], xb[..., 1], onesN,
         (xb * xb).sum(-1).astype(f32)], axis=1
    ) * vb[:, None, :]                                  # [B,4,M]
    # huber term on host (tiny)
    e = (a_pt.sum(axis=1, dtype=f32) - b_pt.sum(axis=1, dtype=f32)).astype(f32)
    hub = np.where(np.abs(e) <= 1.0, f32(0.5) * e * e, np.abs(e) - f32(0.5))
    chunk = lambda x: x.reshape(B, 4, 128).astype(f32)
    AB = np.concatenate([A.astype(f32), Bm.astype(f32)], axis=2)  # [B,4,1024]
    return AB, chunk(aw), chunk(bw), hub.astype(f32)


def kernel(a_mask, pc_a, b_mask, pc_b, _trace=False):
    AB, aw_pm, bw_pm, hub = _host_prep(
        np.asarray(a_mask), np.asarray(pc_a), np.asarray(b_mask), np.asarray(pc_b)
    )
    eye = np.eye(128, dtype=np.float32)
    in_maps = []
    for core in range(N_CORES):
        sl = slice(core * ELEMS, (core + 1) * ELEMS)
        # [p, (c, e)] layout per weight
        cols = [x[sl].transpose(2, 1, 0).reshape(128, 16)
                for x in (aw_pm, bw_pm)]
        in_maps.append({
            "ABaug": np.ascontiguousarray(AB[sl]),
            "wts": np.ascontiguousarray(np.concatenate(cols, axis=1)),
            "eye": eye,
        })
    nc = _get_nc()
    res = run_bass_kernel_spmd(nc, in_maps, list(range(N_CORES)), trace=_trace)
    ot = np.concatenate([res.results[c]["ot"].reshape(ELEMS) for c in range(N_CORES)])
    out = (ot + hub).astype(np.float32)
    if _trace:
        return out, res
    return out


# revision 29
# speedup vs baseline: 1.0123x; 1.0123x over previous
"""Trainium2 Bass kernel for nn_EnergyMovers (batched Sinkhorn OT loss).

Strategy (pure data parallelism, 4 batch elems per core x 8 cores):
  - Host: build masked augmented point vectors so d2[n,m] = sum_k A[k,n]*B[k,m]
    comes out of a K=4 TensorE matmul already masked (masked rows/cols -> d2=0
    -> K=exp(-sqrt(1e-12)/eps) ~ 1, matching the reference's logK=0 there).
  - Device per elem (A-layout only): d2 -> clamp(DVE) -> sqrt(ACT, fused
    [128,4*512] tile) -> exp(ACT, fused) -> raw K. Then:
      K'A = aw_n * K        via ACT Copy-with-per-partition-scale, whose
                            accum_out also yields rowsum(K'A) for the first
                            u-update (v0 = 1 incl. the reference's masked
                            exp(0)=1 columns) for free,
      K'B = bw_m * K^T      via 128x128 PE transposes of raw K + a
                            per-partition-scale DVE evacuation multiply,
      DK' = clamp(d2)*K'A   as 16 small DVE muls sprinkled one-per-phase
                            into the loop's DVE slack (only needed at the
                            final reduction).
  - Non-log Sinkhorn on reciprocal potentials (u = aw*U~, v = bw*V~):
        s_v = K'A^T @ U~ ; V~ = 1/s_v ; s_u = K'B^T @ V~ ; U~ = 1/s_u
    identical to the reference's log-domain iteration (f32/bf16 exponent
    range suffices).
  - Each phase is QUARTER-PIPELINED on the PE: the matvec output is split
    into four [128,128] PSUM quarters; each quarter's 4 contraction bursts
    (4 batch elems column-tiled at tile_position (0,32e), streaming
    concurrently) complete early, so its evacuation (alternating DVE/ACT,
    distinct tiles to avoid Tile's same-PSUM-reader serialization) and its
    selector matmul overlap the later quarters' streams. sel[32e,e]=1
    picks the 4 result rows -> pt16[128,(c,e)] partition-major, then ONE
    strided DVE reciprocal produces the next stationaries. The M=32
    stationary has the potential chunk in col 0, zeros elsewhere
    (zero-fills unused PSUM rows -> evacuations read no garbage).
  - A ~7us dummy-matmul burst right before the loop flips the HAM clock
    gate to 8/8 so the loop's matmuls run at 2.4 GHz; in-loop PE gaps stay
    well under the ~3.4us MID window so it never re-throttles.
  - Final: ot[e] = (DK'^T U~) . (bw * V~) via one more quarter-pipelined
    matvec + selector matmuls + DVE muls + ones-matmul partition reduction.
  - Host: huber(e) added, results gathered from 8 cores.
"""

import os
from contextlib import ExitStack

import numpy as np

import concourse.bass as bass
import concourse.bacc as bacc
import concourse.mybir as mybir
import concourse.tile as tile
from concourse.bass_utils import run_bass_kernel_spmd

N_CORES = 8
ELEMS = 4  # batch elements per core (B=32 / 8)
B, N, M = 32, 512, 512
EPS = 0.05
ITERS = int(os.environ.get("EM_ITERS", "50"))
F32 = mybir.dt.float32
BF16 = mybir.dt.bfloat16
AF = mybir.ActivationFunctionType
ALU = mybir.AluOpType


def _build_nc():
    nc = bacc.Bacc()
    ABaug = nc.declare_dram_parameter("ABaug", [ELEMS, 4, 2 * N],
                                      mybir.dt.float32r, isOutput=False)
    # wts cols: 0:16 aw[(c,e)], 16:32 bw[(c,e)]
    wtsp = nc.declare_dram_parameter("wts", [128, 32], F32, isOutput=False)
    eyep = nc.declare_dram_parameter("eye", [128, 128], F32, isOutput=False)
    otp = nc.declare_dram_parameter("ot", [1, ELEMS], F32, isOutput=True)
    # keeps the HAM warm-up matmuls live through dead-code elim
    scrp = nc.declare_dram_parameter("scr", [1, 1], F32, isOutput=True)

    with ExitStack() as ctx:
        tc = ctx.enter_context(tile.TileContext(nc))
        kpool = ctx.enter_context(tc.tile_pool(name="kmat", bufs=1))
        vpool = ctx.enter_context(tc.tile_pool(name="vec", bufs=1))

        # --- params / constants -----------------------------------------
        wt_sb = vpool.tile([128, 32], F32, tag="wt", name="wt")
        nc.sync.dma_start(out=wt_sb[:], in_=wtsp[:])
        aw_v = wt_sb[:, 0:16].rearrange("p (c e) -> p c e", c=4)
        bw_v = wt_sb[:, 16:32].rearrange("p (c e) -> p c e", c=4)
        eye_sb = vpool.tile([128, 128], F32, tag="eyef", name="eyef")
        nc.sync.dma_start(out=eye_sb[:], in_=eyep[:])
        identB = vpool.tile([128, 128], BF16, tag="identB", name="identB")
        nc.vector.tensor_copy(identB[:], eye_sb[:])
        ones = vpool.tile([128, 1], F32, tag="ones", name="ones")
        nc.gpsimd.memset(ones[:], 1.0)
        sel = vpool.tile([128, 4], BF16, tag="sel", name="sel")
        nc.gpsimd.memset(sel[:], 0.0)
        for e in range(ELEMS):
            nc.gpsimd.memset(sel[32 * e:32 * e + 1, e:e + 1], 1.0)
        outsb = vpool.tile([1, ELEMS], F32, tag="outsb", name="outsb")
        bias12 = vpool.tile([128, 1], F32, tag="bias12", name="bias12")
        nc.gpsimd.memset(bias12[:], 1e-12)

        # potentials: [128, (c, e, 32)] bf16; col 0 of each 32-block is the
        # live value, cols 1-31 stay zero (zero-pads the M=32 stationary).
        U_all = vpool.tile([128, 4, 4, 32], BF16, tag="U", name="U")
        V_all = vpool.tile([128, 4, 4, 32], BF16, tag="V", name="V")
        nc.gpsimd.memset(U_all[:], 0.0)
        nc.gpsimd.memset(V_all[:], 0.0)

        KA, KB, DK, CLA, AB_SB = {}, {}, {}, {}, {}
        for e in range(ELEMS):
            ab_sb = vpool.tile([4, 2 * N], mybir.dt.float32r,
                               tag=f"ABs{e}", name=f"ABs{e}")
            nc.sync.dma_start(out=ab_sb[:], in_=ABaug[e])
            AB_SB[e] = (ab_sb[:, 0:N], ab_sb[:, N:2 * N])
            KA[e] = kpool.tile([128, 4, 512], BF16, tag=f"KA{e}", name=f"KA{e}")
            KB[e] = kpool.tile([128, 4, 512], BF16, tag=f"KB{e}", name=f"KB{e}")
            DK[e] = kpool.tile([128, 4, 512], BF16, tag=f"DK{e}", name=f"DK{e}")
            CLA[e] = kpool.tile([128, 4, 512], F32, tag=f"cl{e}", name=f"cl{e}")

        rs = vpool.tile([128, 16], F32, tag="rs", name="rs")
        last_evac = {}
        with tc.tile_pool(name="pd2", bufs=2, space="PSUM") as pd2, \
             tc.tile_pool(name="ptp", bufs=2, space="PSUM") as ptp, \
             tc.tile_pool(name="kr", bufs=1) as krpool, \
             tc.tile_pool(name="st", bufs=4) as stpool:
            KRAW = {}
            last_sqrt = None
            # d2 (A layout) -> clamp -> sqrt, fused [128,2,512]/[128,4,512]
            for e in range(ELEMS):
                a_sb, b_sb = AB_SB[e]
                for h in range(2):
                    d2 = pd2.tile([128, 2, 512], F32, tag="d2", name="d2")
                    for c2 in range(2):
                        c = 2 * h + c2
                        nc.tensor.matmul(
                            d2[:, c2, :], a_sb[:, c * 128:(c + 1) * 128],
                            b_sb[:], start=True, stop=True,
                        )
                    nc.vector.tensor_scalar_max(
                        CLA[e][:, 2 * h:2 * h + 2, :], d2[:], 0.0
                    )
                st = stpool.tile([128, 4, 512], F32, tag="st", name="st")
                last_sqrt = nc.scalar.activation(
                    st[:], CLA[e][:], AF.Sqrt, bias=bias12[:]
                )
                KRAW[e] = (st, krpool.tile([128, 4, 512], BF16,
                                           tag=f"kr{e}", name=f"kr{e}"))
            # all exps after all sqrts (ACT table sets differ)
            exp_done = {}
            for e in range(ELEMS):
                st, kraw = KRAW[e]
                exp_inst = nc.scalar.activation(
                    kraw[:], st[:], AF.Exp, scale=-1.0 / EPS,
                )
                tile.add_dep_helper(exp_inst.ins, last_sqrt.ins,
                                    sync=True, reason="act-table-batch")
                exp_done[e] = exp_inst
            # K'A = aw * K via ACT Copy-with-scale; accum -> rowsums
            for e in range(ELEMS):
                kraw = KRAW[e][1]
                for c in range(4):
                    nc.scalar.activation(
                        KA[e][:, c, :], kraw[:, c, :], AF.Copy,
                        scale=aw_v[:, c, e:e + 1],
                        accum_out=rs[:, 4 * c + e:4 * c + e + 1],
                    )
            # K'B = bw * K^T via PE transposes + per-partition-scale evac
            for e in range(ELEMS):
                kraw = KRAW[e][1]
                for cm in range(4):
                    tp = ptp.tile([128, 4, 128], BF16, tag="tp", name="tp")
                    for ci in range(4):
                        nc.tensor.transpose(
                            tp[:, ci, :],
                            kraw[:, ci, cm * 128:(cm + 1) * 128], identB[:],
                        )
                    last_evac[(e, cm)] = nc.vector.tensor_scalar_mul(
                        KB[e][:, cm, :],
                        tp[:].rearrange("p a b -> p (a b)"),
                        bw_v[:, cm, e:e + 1],
                    )
            # first u-update: U~1 = aw / rowsum(K'A)  (v0 = 1 incl. masked)
            rs2 = vpool.tile([128, 16], F32, tag="rs2", name="rs2")
            nc.vector.tensor_scalar_max(rs2[:], rs[:], 1e-30)
            rcp = vpool.tile([128, 16], F32, tag="rcp", name="rcp")
            nc.vector.reciprocal(rcp[:], rs2[:])
            nc.vector.tensor_mul(
                U_all[:, :, :, 0],
                rcp[:].rearrange("p (c e) -> p c e", c=4), aw_v,
            )

        # --- Sinkhorn iterations ----------------------------------------
        with tc.tile_pool(name="psq", bufs=1, space="PSUM") as qpool, \
             tc.tile_pool(name="pt16", bufs=2, space="PSUM") as ptpool, \
             tc.tile_pool(name="dps", bufs=1, space="PSUM") as dpool, \
             tc.tile_pool(name="sb", bufs=2) as sbpool:

            dummy_ps = dpool.tile([1, 512], F32, tag="dps", name="dps")
            # warm-up burst: ~7us of back-to-back matmuls flips the HAM
            # clock gate to 8/8 right before the loop (gated so the
            # PE-idle ACT stretch of setup comes first)
            for i in range(16):
                wm = nc.tensor.matmul(
                    dummy_ps[:], sel[:, 0:1], KA[0][:, 0, :],
                    start=True, stop=True, skip_group_check=True,
                )
                if i == 0:
                    tile.add_dep_helper(wm.ins, last_evac[(1, 3)].ins,
                                        sync=True, reason="warmup-after-setup")

            PSQ = [qpool.tile([128, 256], F32, tag=f"q{q}", name=f"q{q}")
                   for q in range(2)]

            def emit_head(Kt, stat):
                # first two h0 chunk-bursts of a phase; only need the c0/c1
                # stationaries (the early reciprocal half, via subtile deps)
                for c in (0, 1):
                    for e in range(ELEMS):
                        nc.tensor.matmul(
                            PSQ[0][32 * e:32 * e + 32, :],
                            stat[:, c, e, :], Kt[e][:, c, 0:256],
                            start=(c == 0), stop=False,
                            tile_position=(0, 32 * e),
                        )

            def filler(n, src):
                # HAM-warmth fillers anchored on this phase's evacuated
                # tile so the scheduler cannot hoist them elsewhere
                for _ in range(n):
                    nc.tensor.matmul(
                        dummy_ps[:, 0:128], sel[:, 0:1], src[:, 0:128],
                        start=True, stop=True, skip_group_check=True,
                    )

            def phase(Kt, stat, out_all, nxt, head_done):
                """out_all[:,c,e,0] = 1 / (sum_c stat[:,c,e,:]^T @ Kt).
                Software-pipelined: the h0 c0/c1 bursts may have been
                pre-emitted into the previous phase's tail; this phase
                pre-emits nxt's in turn. Both evacuations run on ACT so
                the DVE only carries the critical-path reciprocals."""
                if not head_done:
                    emit_head(Kt, stat)
                for c in (2, 3):
                    for e in range(ELEMS):
                        nc.tensor.matmul(
                            PSQ[0][32 * e:32 * e + 32, :],
                            stat[:, c, e, :], Kt[e][:, c, 0:256],
                            start=False, stop=(c == 3),
                            tile_position=(0, 32 * e),
                        )
                sbq0 = sbpool.tile([128, 256], BF16, tag="sbq0", name="sbq0")
                nc.scalar.copy(sbq0[:], PSQ[0][:])
                for c in range(4):
                    for e in range(ELEMS):
                        nc.tensor.matmul(
                            PSQ[1][32 * e:32 * e + 32, :],
                            stat[:, c, e, :], Kt[e][:, c, 256:512],
                            start=(c == 0), stop=(c == 3),
                            tile_position=(0, 32 * e),
                        )
                sbq1 = sbpool.tile([128, 256], BF16, tag="sbq1", name="sbq1")
                nc.scalar.copy(sbq1[:], PSQ[1][:])
                SBQ = [sbq0, sbq1]
                pt16 = ptpool.tile([128, 4, 4], F32, tag="pt16", name="pt16")
                with nc.allow_low_precision("bf16 Sinkhorn potentials"):
                    for c in (0, 1):
                        nc.tensor.matmul(
                            pt16[:, c, :], sbq0[:, c * 128:(c + 1) * 128],
                            sel[:], start=True, stop=True,
                        )
                    filler(1, sbq0)
                    nc.vector.reciprocal(out_all[:, 0:2, :, 0],
                                         pt16[:, 0:2, :])
                    for c in (2, 3):
                        nc.tensor.matmul(
                            pt16[:, c, :],
                            sbq1[:, (c - 2) * 128:(c - 1) * 128],
                            sel[:], start=True, stop=True,
                        )
                    if nxt is not None:
                        # runs on the PE while recip-b executes on the DVE
                        emit_head(nxt[0], nxt[1])
                    else:
                        filler(2, sbq1)
                    nc.vector.reciprocal(out_all[:, 2:4, :, 0],
                                         pt16[:, 2:4, :])
                return nxt is not None

            # reference order: 50x(u-update; v-update). u#1 done at setup.
            # DK' = clamp(d2)*K'A muls hide in the loop's DVE slack.
            dk_jobs = [(e, c) for e in range(ELEMS) for c in range(4)]
            jobs = []
            for _ in range(ITERS - 1):
                jobs.append((KA, U_all, V_all))
                jobs.append((KB, V_all, U_all))
            jobs.append((KA, U_all, V_all))
            head_done = False
            nphase = 0
            for i, (Kt, stat, out_all) in enumerate(jobs[:-1]):
                nxt = jobs[i + 1]
                head_done = phase(Kt, stat, out_all, nxt, head_done)
                nphase += 1
                if nphase - 1 < len(dk_jobs):
                    e, c = dk_jobs[nphase - 1]
                    nc.vector.tensor_mul(
                        DK[e][:, c, :], CLA[e][:, c, :], KA[e][:, c, :]
                    )
            for e, c in dk_jobs[nphase:]:  # leftovers (small ITERS only)
                nc.vector.tensor_mul(
                    DK[e][:, c, :], CLA[e][:, c, :], KA[e][:, c, :]
                )
            Kt, stat, out_all = jobs[-1]
            phase(Kt, stat, out_all, None, head_done)  # final v-update

            # --- final: ot[e] = (DK'^T U~) . (bw * V~) ------------------
            SBG = []
            for h in range(2):
                for c in range(4):
                    for e in range(ELEMS):
                        nc.tensor.matmul(
                            PSQ[h][32 * e:32 * e + 32, :],
                            U_all[:, c, e, :],
                            DK[e][:, c, 256 * h:256 * h + 256],
                            start=(c == 0), stop=(c == 3),
                            tile_position=(0, 32 * e),
                        )
                sbg = sbpool.tile([128, 256], BF16, tag=f"sbq{h}",
                                  name=f"sbg{h}")
                if h == 0:
                    nc.vector.tensor_copy(sbg[:], PSQ[h][:])
                else:
                    nc.scalar.copy(sbg[:], PSQ[h][:])
                SBG.append(sbg)
            ptg = ptpool.tile([128, 4, 4], F32, tag="pt16", name="ptg")
            for c in range(4):
                nc.tensor.matmul(
                    ptg[:, c, :],
                    SBG[c // 2][:, (c % 2) * 128:(c % 2 + 1) * 128],
                    sel[:], start=True, stop=True,
                )
            t1 = sbpool.tile([128, 4, 4], F32, tag="t1", name="t1")
            nc.vector.tensor_mul(t1[:], ptg[:], V_all[:, :, :, 0])
            t2 = sbpool.tile([128, 4, 4], F32, tag="t2", name="t2")
            nc.vector.tensor_mul(t2[:], t1[:], bw_v)
            r_e = sbpool.tile([128, 4], F32, tag="re", name="re")
            nc.vector.reduce_sum(
                r_e[:], t2[:].rearrange("p c e -> p e c"),
                axis=mybir.AxisListType.X,
            )
            po = ptpool.tile([1, 4], F32, tag="pt16", name="po")
            nc.tensor.matmul(po[:], ones[:], r_e[:], start=True, stop=True)
            nc.scalar.copy(outsb[:], po[:])
            nc.sync.dma_start(out=otp[:], in_=outsb[:])
            # keep the warm-up matmuls live
            scr_sb = vpool.tile([1, 1], F32, tag="scr", name="scr")
            nc.scalar.copy(scr_sb[:], dummy_ps[0:1, 0:1])
            nc.sync.dma_start(out=scrp[:], in_=scr_sb[:])
    nc.compile()
    return nc


_NC_CACHE = {}


def _get_nc():
    if "nc" not in _NC_CACHE:
        _NC_CACHE["nc"] = _build_nc()
    return _NC_CACHE["nc"]


def _host_prep(a_mask, pc_a, b_mask, pc_b):
    """Per-batch-element f32 prep mirroring the reference's masking."""
    f32 = np.float32
    a_pt = (a_mask * pc_a[..., 2]).astype(f32)          # [B,N]
    b_pt = (b_mask * pc_b[..., 2]).astype(f32)          # [B,M]
    va = (a_pt > 0).astype(f32)
    vb = (b_pt > 0).astype(f32)
    aw = (a_pt / a_pt.sum(axis=1, keepdims=True, dtype=f32)).astype(f32)
    bw = (b_pt / b_pt.sum(axis=1, keepdims=True, dtype=f32)).astype(f32)
    xa = pc_a[..., :2].astype(f32)                      # [B,N,2]
    xb = pc_b[..., :2].astype(f32)
    onesN = np.ones((B, N), f32)
    A = np.stack(
        [-2 * xa[..., 0], -2 * xa[..., 1],
         (xa * xa).sum(-1).astype(f32), onesN], axis=1
    ) * va[:, None, :]                                  # [B,4,N]
    Bm = np.stack(
        [xb[..., 0], xb[..., 1], onesN,
         (xb * xb).sum(-1).astype(f32)], axis=1
    ) * vb[:, None, :]                                  # [B,4,M]
    # huber term on host (tiny)
    e = (a_pt.sum(axis=1, dtype=f32) - b_pt.sum(axis=1, dtype=f32)).astype(f32)
    hub = np.where(np.abs(e) <= 1.0, f32(0.5) * e * e, np.abs(e) - f32(0.5))
    chunk = lambda x: x.reshape(B, 4, 128).astype(f32)
    AB = np.concatenate([A.astype(f32), Bm.astype(f32)], axis=2)  # [B,4,1024]
    return AB, chunk(aw), chunk(bw), hub.astype(f32)


def kernel(a_mask, pc_a, b_mask, pc_b, _trace=False):
    AB, aw_pm, bw_pm, hub = _host_prep(
        np.asarray(a_mask), np.asarray(pc_a), np.asarray(b_mask), np.asarray(pc_b)
    )
    eye = np.eye(128, dtype=np.float32)
    in_maps = []
    for core in range(N_CORES):
        sl = slice(core * ELEMS, (core + 1) * ELEMS)
        # [p, (c, e)] layout per weight
        cols = [x[sl].transpose(2, 1, 0).reshape(128, 16)
                for x in (aw_pm, bw_pm)]
        in_maps.append({
            "ABaug": np.ascontiguousarray(AB[sl]),
            "wts": np.ascontiguousarray(np.concatenate(cols, axis=1)),
            "eye": eye,
        })
    nc = _get_nc()
    res = run_bass_kernel_spmd(nc, in_maps, list(range(N_CORES)), trace=_trace)
    ot = np.concatenate([res.results[c]["ot"].reshape(ELEMS) for c in range(N_CORES)])
    out = (ot + hub).astype(np.float32)
    if _trace:
        return out, res
    return out


# revision 30
# speedup vs baseline: 1.0705x; 1.0574x over previous
"""Trainium2 Bass kernel for nn_EnergyMovers (batched Sinkhorn OT loss).

Strategy (pure data parallelism, 4 batch elems per core x 8 cores):
  - Host: build masked augmented point vectors so d2[n,m] = sum_k A[k,n]*B[k,m]
    comes out of a K=4 TensorE matmul already masked (masked rows/cols -> d2=0
    -> K=exp(-sqrt(1e-12)/eps) ~ 1, matching the reference's logK=0 there).
  - Device per elem (A-layout only): d2 -> clamp(DVE) -> sqrt(ACT, fused
    [128,4*512] tile) -> exp(ACT, fused) -> raw K. Then:
      K'A = aw_n * K        via ACT Copy-with-per-partition-scale, whose
                            accum_out also yields rowsum(K'A) for the first
                            u-update (v0 = 1 incl. the reference's masked
                            exp(0)=1 columns) for free,
      K'B = bw_m * K^T      via 128x128 PE transposes of raw K + a
                            per-partition-scale DVE evacuation multiply,
      DK' = clamp(d2)*K'A   as 16 small DVE muls sprinkled one-per-phase
                            into the loop's DVE slack (only needed at the
                            final reduction).
  - Non-log Sinkhorn on reciprocal potentials (u = aw*U~, v = bw*V~):
        s_v = K'A^T @ U~ ; V~ = 1/s_v ; s_u = K'B^T @ V~ ; U~ = 1/s_u
    identical to the reference's log-domain iteration (f32/bf16 exponent
    range suffices).
  - Each phase is QUARTER-PIPELINED on the PE: the matvec output is split
    into four [128,128] PSUM quarters; each quarter's 4 contraction bursts
    (4 batch elems column-tiled at tile_position (0,32e), streaming
    concurrently) complete early, so its evacuation (alternating DVE/ACT,
    distinct tiles to avoid Tile's same-PSUM-reader serialization) and its
    selector matmul overlap the later quarters' streams. sel[32e,e]=1
    picks the 4 result rows -> pt16[128,(c,e)] partition-major, then ONE
    strided DVE reciprocal produces the next stationaries. The M=32
    stationary has the potential chunk in col 0, zeros elsewhere
    (zero-fills unused PSUM rows -> evacuations read no garbage).
  - A ~7us dummy-matmul burst right before the loop flips the HAM clock
    gate to 8/8 so the loop's matmuls run at 2.4 GHz; in-loop PE gaps stay
    well under the ~3.4us MID window so it never re-throttles.
  - Final: ot[e] = (DK'^T U~) . (bw * V~) via one more quarter-pipelined
    matvec + selector matmuls + DVE muls + ones-matmul partition reduction.
  - Host: huber(e) added, results gathered from 8 cores.
"""

import os
from contextlib import ExitStack

import numpy as np

import concourse.bass as bass
import concourse.bacc as bacc
import concourse.mybir as mybir
import concourse.tile as tile
from concourse.bass_utils import run_bass_kernel_spmd

N_CORES = 8
ELEMS = 4  # batch elements per core (B=32 / 8)
B, N, M = 32, 512, 512
EPS = 0.05
ITERS = int(os.environ.get("EM_ITERS", "50"))
F32 = mybir.dt.float32
BF16 = mybir.dt.bfloat16
AF = mybir.ActivationFunctionType
ALU = mybir.AluOpType


def _build_nc():
    nc = bacc.Bacc()
    ABaug = nc.declare_dram_parameter("ABaug", [ELEMS, 4, 2 * N],
                                      mybir.dt.float32r, isOutput=False)
    # wts cols: 0:16 aw[(c,e)], 16:32 bw[(c,e)]
    wtsp = nc.declare_dram_parameter("wts", [128, 32], F32, isOutput=False)
    eyep = nc.declare_dram_parameter("eye", [128, 128], F32, isOutput=False)
    otp = nc.declare_dram_parameter("ot", [1, ELEMS], F32, isOutput=True)
    # keeps the HAM warm-up matmuls live through dead-code elim
    scrp = nc.declare_dram_parameter("scr", [1, 1], F32, isOutput=True)

    with ExitStack() as ctx:
        tc = ctx.enter_context(tile.TileContext(nc))
        kpool = ctx.enter_context(tc.tile_pool(name="kmat", bufs=1))
        vpool = ctx.enter_context(tc.tile_pool(name="vec", bufs=1))

        # --- params / constants -----------------------------------------
        wt_sb = vpool.tile([128, 32], F32, tag="wt", name="wt")
        nc.sync.dma_start(out=wt_sb[:], in_=wtsp[:])
        aw_v = wt_sb[:, 0:16].rearrange("p (c e) -> p c e", c=4)
        bw_v = wt_sb[:, 16:32].rearrange("p (c e) -> p c e", c=4)
        eye_sb = vpool.tile([128, 128], F32, tag="eyef", name="eyef")
        nc.sync.dma_start(out=eye_sb[:], in_=eyep[:])
        identB = vpool.tile([128, 128], BF16, tag="identB", name="identB")
        nc.vector.tensor_copy(identB[:], eye_sb[:])
        ones = vpool.tile([128, 1], F32, tag="ones", name="ones")
        nc.gpsimd.memset(ones[:], 1.0)
        sel = vpool.tile([128, 4], BF16, tag="sel", name="sel")
        nc.gpsimd.memset(sel[:], 0.0)
        for e in range(ELEMS):
            nc.gpsimd.memset(sel[32 * e:32 * e + 1, e:e + 1], 1.0)
        outsb = vpool.tile([1, ELEMS], F32, tag="outsb", name="outsb")
        bias12 = vpool.tile([128, 1], F32, tag="bias12", name="bias12")
        nc.gpsimd.memset(bias12[:], 1e-12)

        # potentials: [128, (c, e, 32)] bf16; col 0 of each 32-block is the
        # live value, cols 1-31 stay zero (zero-pads the M=32 stationary).
        U_all = vpool.tile([128, 4, 4, 32], BF16, tag="U", name="U")
        V_all = vpool.tile([128, 4, 4, 32], BF16, tag="V", name="V")
        nc.gpsimd.memset(U_all[:], 0.0)
        nc.gpsimd.memset(V_all[:], 0.0)

        KA, KB, DK, CLA, AB_SB = {}, {}, {}, {}, {}
        for e in range(ELEMS):
            ab_sb = vpool.tile([4, 2 * N], mybir.dt.float32r,
                               tag=f"ABs{e}", name=f"ABs{e}")
            nc.sync.dma_start(out=ab_sb[:], in_=ABaug[e])
            AB_SB[e] = (ab_sb[:, 0:N], ab_sb[:, N:2 * N])
            KA[e] = kpool.tile([128, 4, 512], BF16, tag=f"KA{e}", name=f"KA{e}")
            KB[e] = kpool.tile([128, 4, 512], BF16, tag=f"KB{e}", name=f"KB{e}")
            DK[e] = kpool.tile([128, 4, 512], BF16, tag=f"DK{e}", name=f"DK{e}")
            CLA[e] = kpool.tile([128, 4, 512], F32, tag=f"cl{e}", name=f"cl{e}")

        rs = vpool.tile([128, 16], F32, tag="rs", name="rs")
        last_evac = {}
        with tc.tile_pool(name="pd2", bufs=2, space="PSUM") as pd2, \
             tc.tile_pool(name="ptp", bufs=2, space="PSUM") as ptp, \
             tc.tile_pool(name="kr", bufs=1) as krpool, \
             tc.tile_pool(name="st", bufs=4) as stpool:
            KRAW = {}
            last_sqrt = None
            # d2 (A layout) -> clamp -> sqrt, fused [128,2,512]/[128,4,512]
            for e in range(ELEMS):
                a_sb, b_sb = AB_SB[e]
                for h in range(2):
                    d2 = pd2.tile([128, 2, 512], F32, tag="d2", name="d2")
                    for c2 in range(2):
                        c = 2 * h + c2
                        nc.tensor.matmul(
                            d2[:, c2, :], a_sb[:, c * 128:(c + 1) * 128],
                            b_sb[:], start=True, stop=True,
                        )
                    nc.vector.tensor_scalar_max(
                        CLA[e][:, 2 * h:2 * h + 2, :], d2[:], 0.0
                    )
                st = stpool.tile([128, 4, 512], F32, tag="st", name="st")
                last_sqrt = nc.scalar.activation(
                    st[:], CLA[e][:], AF.Sqrt, bias=bias12[:]
                )
                KRAW[e] = (st, krpool.tile([128, 4, 512], BF16,
                                           tag=f"kr{e}", name=f"kr{e}"))
            # all exps after all sqrts (ACT table sets differ)
            exp_done = {}
            for e in range(ELEMS):
                st, kraw = KRAW[e]
                exp_inst = nc.scalar.activation(
                    kraw[:], st[:], AF.Exp, scale=-1.0 / EPS,
                )
                tile.add_dep_helper(exp_inst.ins, last_sqrt.ins,
                                    sync=True, reason="act-table-batch")
                exp_done[e] = exp_inst
            # K'A = aw * K via ACT Copy-with-scale; accum -> rowsums
            for e in range(ELEMS):
                kraw = KRAW[e][1]
                for c in range(4):
                    nc.scalar.activation(
                        KA[e][:, c, :], kraw[:, c, :], AF.Copy,
                        scale=aw_v[:, c, e:e + 1],
                        accum_out=rs[:, 4 * c + e:4 * c + e + 1],
                    )
            # K'B = bw * K^T via PE transposes + per-partition-scale evac
            for e in range(ELEMS):
                kraw = KRAW[e][1]
                for cm in range(4):
                    tp = ptp.tile([128, 4, 128], BF16, tag="tp", name="tp")
                    for ci in range(4):
                        nc.tensor.transpose(
                            tp[:, ci, :],
                            kraw[:, ci, cm * 128:(cm + 1) * 128], identB[:],
                        )
                    last_evac[(e, cm)] = nc.vector.tensor_scalar_mul(
                        KB[e][:, cm, :],
                        tp[:].rearrange("p a b -> p (a b)"),
                        bw_v[:, cm, e:e + 1],
                    )
            # first u-update: U~1 = aw / rowsum(K'A)  (v0 = 1 incl. masked)
            rs2 = vpool.tile([128, 16], F32, tag="rs2", name="rs2")
            nc.vector.tensor_scalar_max(rs2[:], rs[:], 1e-30)
            rcp = vpool.tile([128, 16], F32, tag="rcp", name="rcp")
            nc.vector.reciprocal(rcp[:], rs2[:])
            nc.vector.tensor_mul(
                U_all[:, :, :, 0],
                rcp[:].rearrange("p (c e) -> p c e", c=4), aw_v,
            )

        # --- Sinkhorn iterations ----------------------------------------
        with tc.tile_pool(name="psq", bufs=1, space="PSUM") as qpool, \
             tc.tile_pool(name="pt16", bufs=2, space="PSUM") as ptpool, \
             tc.tile_pool(name="dps", bufs=1, space="PSUM") as dpool, \
             tc.tile_pool(name="sb", bufs=2) as sbpool:

            dummy_ps = dpool.tile([1, 512], F32, tag="dps", name="dps")
            # warm-up burst: ~7us of back-to-back matmuls flips the HAM
            # clock gate to 8/8 right before the loop (gated so the
            # PE-idle ACT stretch of setup comes first)
            for i in range(16):
                wm = nc.tensor.matmul(
                    dummy_ps[:], sel[:, 0:1], KA[0][:, 0, :],
                    start=True, stop=True, skip_group_check=True,
                )
                if i == 0:
                    tile.add_dep_helper(wm.ins, last_evac[(1, 3)].ins,
                                        sync=True, reason="warmup-after-setup")

            PSQ = [qpool.tile([128, 256], F32, tag=f"q{q}", name=f"q{q}")
                   for q in range(2)]

            def phase(Kt, stat, out_all):
                """out_all[:,c,e,0] = 1 / (sum_c stat[:,c,e,:]^T @ Kt)"""
                SBQ = []
                for h in range(2):
                    for c in range(4):
                        for e in range(ELEMS):
                            nc.tensor.matmul(
                                PSQ[h][32 * e:32 * e + 32, :],
                                stat[:, c, e, :],
                                Kt[e][:, c, 256 * h:256 * h + 256],
                                start=(c == 0), stop=(c == 3),
                                tile_position=(0, 32 * e),
                            )
                    sbq = sbpool.tile([128, 256], BF16, tag=f"sbq{h}",
                                      name=f"sbq{h}")
                    if h == 0:
                        # ACT (slower) gets the early half: hides under the
                        # second half's matmul streams
                        nc.scalar.copy(sbq[:], PSQ[h][:])
                    else:
                        nc.vector.tensor_copy(sbq[:], PSQ[h][:])
                    SBQ.append(sbq)

                def filler(n, src):
                    # HAM-warmth fillers anchored on this phase's evacuated
                    # tile so the scheduler cannot hoist them out of the
                    # tail's wait windows
                    for _ in range(n):
                        nc.tensor.matmul(
                            dummy_ps[:, 0:128], sel[:, 0:1], src[:, 0:128],
                            start=True, stop=True, skip_group_check=True,
                        )

                pt16 = ptpool.tile([128, 4, 4], F32, tag="pt16", name="pt16")
                with nc.allow_low_precision("bf16 Sinkhorn potentials"):
                    for c in range(4):
                        nc.tensor.matmul(
                            pt16[:, c, :],
                            SBQ[c // 2][:, (c % 2) * 128:(c % 2 + 1) * 128],
                            sel[:], start=True, stop=True,
                        )
                        if c == 1:
                            filler(3, SBQ[0])
                            # early half: unblocks next phase's c0/c1 bursts
                            # (subtile deps) while sel2/sel3 still run
                            nc.vector.reciprocal(out_all[:, 0:2, :, 0],
                                                 pt16[:, 0:2, :])
                    filler(3, SBQ[1])
                    nc.vector.reciprocal(out_all[:, 2:4, :, 0],
                                         pt16[:, 2:4, :])

            # reference order: 50x(u-update; v-update). u#1 done at setup.
            # DK' = clamp(d2)*K'A muls hide in the loop's DVE slack.
            dk_jobs = [(e, c) for e in range(ELEMS) for c in range(4)]
            nphase = 0
            for _ in range(ITERS - 1):
                for Kt, stat, out_all in ((KA, U_all, V_all),
                                          (KB, V_all, U_all)):
                    phase(Kt, stat, out_all)
                    nphase += 1
                    if nphase - 1 < len(dk_jobs):
                        e, c = dk_jobs[nphase - 1]
                        nc.vector.tensor_mul(
                            DK[e][:, c, :], CLA[e][:, c, :], KA[e][:, c, :]
                        )
            for e, c in dk_jobs[nphase:]:  # leftovers (small ITERS only)
                nc.vector.tensor_mul(
                    DK[e][:, c, :], CLA[e][:, c, :], KA[e][:, c, :]
                )
            phase(KA, U_all, V_all)       # final v-update

            # --- final: ot[e] = (DK'^T U~) . (bw * V~) ------------------
            SBG = []
            for h in range(2):
                for c in range(4):
                    for e in range(ELEMS):
                        nc.tensor.matmul(
                            PSQ[h][32 * e:32 * e + 32, :],
                            U_all[:, c, e, :],
                            DK[e][:, c, 256 * h:256 * h + 256],
                            start=(c == 0), stop=(c == 3),
                            tile_position=(0, 32 * e),
                        )
                sbg = sbpool.tile([128, 256], BF16, tag=f"sbq{h}",
                                  name=f"sbg{h}")
                if h == 0:
                    nc.vector.tensor_copy(sbg[:], PSQ[h][:])
                else:
                    nc.scalar.copy(sbg[:], PSQ[h][:])
                SBG.append(sbg)
            ptg = ptpool.tile([128, 4, 4], F32, tag="pt16", name="ptg")
            for c in range(4):
                nc.tensor.matmul(
                    ptg[:, c, :],
                    SBG[c // 2][:, (c % 2) * 128:(c % 2 + 1) * 128],
                    sel[:], start=True, stop=True,
                )
            t1 = sbpool.tile([128, 4, 4], F32, tag="t1", name="t1")
            nc.vector.tensor_mul(t1[:], ptg[:], V_all[:, :, :, 0])
            t2 = sbpool.tile([128, 4, 4], F32, tag="t2", name="t2")
            nc.vector.tensor_mul(t2[:], t1[:], bw_v)
            r_e = sbpool.tile([128, 4], F32, tag="re", name="re")
            nc.vector.reduce_sum(
                r_e[:], t2[:].rearrange("p c e -> p e c"),
                axis=mybir.AxisListType.X,
            )
            po = ptpool.tile([1, 4], F32, tag="pt16", name="po")
            nc.tensor.matmul(po[:], ones[:], r_e[:], start=True, stop=True)
            nc.scalar.copy(outsb[:], po[:])
            nc.sync.dma_start(out=otp[:], in_=outsb[:])
            # keep the warm-up matmuls live
            scr_sb = vpool.tile([1, 1], F32, tag="scr", name="scr")
            nc.scalar.copy(scr_sb[:], dummy_ps[0:1, 0:1])
            nc.sync.dma_start(out=scrp[:], in_=scr_sb[:])
    nc.compile()
    return nc


_NC_CACHE = {}


def _get_nc():
    if "nc" not in _NC_CACHE:
        _NC_CACHE["nc"] = _build_nc()
    return _NC_CACHE["nc"]


def _host_prep(a_mask, pc_a, b_mask, pc_b):
    """Per-batch-element f32 prep mirroring the reference's masking."""
    f32 = np.float32
    a_pt = (a_mask * pc_a[..., 2]).astype(f32)          # [B,N]
    b_pt = (b_mask * pc_b[..., 2]).astype(f32)          # [B,M]
    va = (a_pt > 0).astype(f32)
    vb = (b_pt > 0).astype(f32)
    aw = (a_pt / a_pt.sum(axis=1, keepdims=True, dtype=f32)).astype(f32)
    bw = (b_pt / b_pt.sum(axis=1, keepdims=True, dtype=f32)).astype(f32)
    xa = pc_a[..., :2].astype(f32)                      # [B,N,2]
    xb = pc_b[..., :2].astype(f32)
    onesN = np.ones((B, N), f32)
    A = np.stack(
        [-2 * xa[..., 0], -2 * xa[..., 1],
         (xa * xa).sum(-1).astype(f32), onesN], axis=1
    ) * va[:, None, :]                                  # [B,4,N]
    Bm = np.stack(
        [xb[..., 0], xb[..., 1], onesN,
         (xb * xb).sum(-1).astype(f32)], axis=1
    ) * vb[:, None, :]                                  # [B,4,M]
    # huber term on host (tiny)
    e = (a_pt.sum(axis=1, dtype=f32) - b_pt.sum(axis=1, dtype=f32)).astype(f32)
    hub = np.where(np.abs(e) <= 1.0, f32(0.5) * e * e, np.abs(e) - f32(0.5))
    chunk = lambda x: x.reshape(B, 4, 128).astype(f32)
    AB = np.concatenate([A.astype(f32), Bm.astype(f32)], axis=2)  # [B,4,1024]
    return AB, chunk(aw), chunk(bw), hub.astype(f32)


def kernel(a_mask, pc_a, b_mask, pc_b, _trace=False):
    AB, aw_pm, bw_pm, hub = _host_prep(
        np.asarray(a_mask), np.asarray(pc_a), np.asarray(b_mask), np.asarray(pc_b)
    )
    eye = np.eye(128, dtype=np.float32)
    in_maps = []
    for core in range(N_CORES):
        sl = slice(core * ELEMS, (core + 1) * ELEMS)
        # [p, (c, e)] layout per weight
        cols = [x[sl].transpose(2, 1, 0).reshape(128, 16)
                for x in (aw_pm, bw_pm)]
        in_maps.append({
            "ABaug": np.ascontiguousarray(AB[sl]),
            "wts": np.ascontiguousarray(np.concatenate(cols, axis=1)),
            "eye": eye,
        })
    nc = _get_nc()
    res = run_bass_kernel_spmd(nc, in_maps, list(range(N_CORES)), trace=_trace)
    ot = np.concatenate([res.results[c]["ot"].reshape(ELEMS) for c in range(N_CORES)])
    out = (ot + hub).astype(np.float32)
    if _trace:
        return out, res
    return out


# revision 35
# speedup vs baseline: 1.1131x; 1.0398x over previous
"""Trainium2 Bass kernel for nn_EnergyMovers (batched Sinkhorn OT loss).

Strategy (pure data parallelism, 4 batch elems per core x 8 cores):
  - Host: build masked augmented point vectors so d2[n,m] = sum_k A[k,n]*B[k,m]
    comes out of a K=4 TensorE matmul already masked (masked rows/cols -> d2=0
    -> K=exp(-sqrt(1e-12)/eps) ~ 1, matching the reference's logK=0 there).
  - Device per elem (A-layout only): d2 -> clamp(DVE) -> sqrt(ACT, fused
    [128,4*512] tile) -> exp(ACT, fused) -> raw K. Then:
      K'A = aw_n * K        via ACT Copy-with-per-partition-scale, whose
                            accum_out also yields rowsum(K'A) for the first
                            u-update (v0 = 1 incl. the reference's masked
                            exp(0)=1 columns) for free,
      K'B = bw_m * K^T      via 128x128 PE transposes of raw K + a
                            per-partition-scale DVE evacuation multiply,
      DK' = clamp(d2)*K'A   as 16 small DVE muls sprinkled one-per-phase
                            into the loop's DVE slack (only needed at the
                            final reduction).
  - Non-log Sinkhorn on reciprocal potentials (u = aw*U~, v = bw*V~):
        s_v = K'A^T @ U~ ; V~ = 1/s_v ; s_u = K'B^T @ V~ ; U~ = 1/s_u
    identical to the reference's log-domain iteration (f32/bf16 exponent
    range suffices).
  - Each phase is QUARTER-PIPELINED on the PE: the matvec output is split
    into four [128,128] PSUM quarters; each quarter's 4 contraction bursts
    (4 batch elems column-tiled at tile_position (0,32e), streaming
    concurrently) complete early, so its evacuation (alternating DVE/ACT,
    distinct tiles to avoid Tile's same-PSUM-reader serialization) and its
    selector matmul overlap the later quarters' streams. sel[32e,e]=1
    picks the 4 result rows -> pt16[128,(c,e)] partition-major, then ONE
    strided DVE reciprocal produces the next stationaries. The M=32
    stationary has the potential chunk in col 0, zeros elsewhere
    (zero-fills unused PSUM rows -> evacuations read no garbage).
  - A ~7us dummy-matmul burst right before the loop flips the HAM clock
    gate to 8/8 so the loop's matmuls run at 2.4 GHz; in-loop PE gaps stay
    well under the ~3.4us MID window so it never re-throttles.
  - Final: ot[e] = (DK'^T U~) . (bw * V~) via one more quarter-pipelined
    matvec + selector matmuls + DVE muls + ones-matmul partition reduction.
  - Host: huber(e) added, results gathered from 8 cores.
"""

import os
from contextlib import ExitStack

import numpy as np

import concourse.bass as bass
import concourse.bacc as bacc
import concourse.mybir as mybir
import concourse.tile as tile
from concourse.bass_utils import run_bass_kernel_spmd

N_CORES = 8
ELEMS = 4  # batch elements per core (B=32 / 8)
B, N, M = 32, 512, 512
EPS = 0.05
ITERS = int(os.environ.get("EM_ITERS", "50"))
F32 = mybir.dt.float32
BF16 = mybir.dt.bfloat16
AF = mybir.ActivationFunctionType
ALU = mybir.AluOpType


def _build_nc():
    nc = bacc.Bacc()
    ABaug = nc.declare_dram_parameter("ABaug", [ELEMS, 4, 2 * N],
                                      mybir.dt.float32r, isOutput=False)
    # wts cols: 0:16 aw[(c,e)], 16:32 bw[(c,e)]
    wtsp = nc.declare_dram_parameter("wts", [128, 32], F32, isOutput=False)
    eyep = nc.declare_dram_parameter("eye", [128, 128], F32, isOutput=False)
    otp = nc.declare_dram_parameter("ot", [1, ELEMS], F32, isOutput=True)
    # keeps the HAM warm-up matmuls live through dead-code elim
    scrp = nc.declare_dram_parameter("scr", [1, 1], F32, isOutput=True)

    with ExitStack() as ctx:
        tc = ctx.enter_context(tile.TileContext(nc))
        kpool = ctx.enter_context(tc.tile_pool(name="kmat", bufs=1))
        vpool = ctx.enter_context(tc.tile_pool(name="vec", bufs=1))

        # --- params / constants -----------------------------------------
        wt_sb = vpool.tile([128, 32], F32, tag="wt", name="wt")
        nc.sync.dma_start(out=wt_sb[:], in_=wtsp[:])
        aw_v = wt_sb[:, 0:16].rearrange("p (c e) -> p c e", c=4)
        bw_v = wt_sb[:, 16:32].rearrange("p (c e) -> p c e", c=4)
        eye_sb = vpool.tile([128, 128], F32, tag="eyef", name="eyef")
        nc.sync.dma_start(out=eye_sb[:], in_=eyep[:])
        identB = vpool.tile([128, 128], BF16, tag="identB", name="identB")
        nc.vector.tensor_copy(identB[:], eye_sb[:])
        ones = vpool.tile([128, 1], F32, tag="ones", name="ones")
        nc.gpsimd.memset(ones[:], 1.0)
        sel = vpool.tile([128, 4], BF16, tag="sel", name="sel")
        nc.gpsimd.memset(sel[:], 0.0)
        for e in range(ELEMS):
            nc.gpsimd.memset(sel[32 * e:32 * e + 1, e:e + 1], 1.0)
        outsb = vpool.tile([1, ELEMS], F32, tag="outsb", name="outsb")
        bias12 = vpool.tile([128, 1], F32, tag="bias12", name="bias12")
        nc.gpsimd.memset(bias12[:], 1e-12)

        # potentials: [128, (c, e, 32)] bf16; col 0 of each 32-block is the
        # live value, cols 1-31 stay zero (zero-pads the M=32 stationary).
        U_all = vpool.tile([128, 4, 4, 32], BF16, tag="U", name="U")
        V_all = vpool.tile([128, 4, 4, 32], BF16, tag="V", name="V")
        nc.gpsimd.memset(U_all[:], 0.0)
        nc.gpsimd.memset(V_all[:], 0.0)

        KA, KB, DK, CLA, AB_SB = {}, {}, {}, {}, {}
        for e in range(ELEMS):
            ab_sb = vpool.tile([4, 2 * N], mybir.dt.float32r,
                               tag=f"ABs{e}", name=f"ABs{e}")
            nc.sync.dma_start(out=ab_sb[:], in_=ABaug[e])
            AB_SB[e] = (ab_sb[:, 0:N], ab_sb[:, N:2 * N])
            KA[e] = kpool.tile([128, 4, 512], BF16, tag=f"KA{e}", name=f"KA{e}")
            KB[e] = kpool.tile([128, 4, 512], BF16, tag=f"KB{e}", name=f"KB{e}")
            DK[e] = kpool.tile([128, 4, 512], BF16, tag=f"DK{e}", name=f"DK{e}")
            CLA[e] = kpool.tile([128, 4, 512], F32, tag=f"cl{e}", name=f"cl{e}")

        rs = vpool.tile([128, 16], F32, tag="rs", name="rs")
        last_evac = {}
        with tc.tile_pool(name="pd2", bufs=3, space="PSUM") as pd2, \
             tc.tile_pool(name="ptp", bufs=2, space="PSUM") as ptp, \
             tc.tile_pool(name="kr", bufs=1) as krpool, \
             tc.tile_pool(name="st", bufs=4) as stpool:
            KRAW = {}
            last_sqrt = None
            # d2 (A layout) -> clamp -> sqrt, fused [128,2,512]/[128,4,512]
            for e in range(ELEMS):
                a_sb, b_sb = AB_SB[e]
                for h in range(2):
                    d2 = pd2.tile([128, 2, 512], F32, tag="d2", name="d2")
                    for c2 in range(2):
                        c = 2 * h + c2
                        nc.tensor.matmul(
                            d2[:, c2, :], a_sb[:, c * 128:(c + 1) * 128],
                            b_sb[:], start=True, stop=True,
                        )
                    nc.vector.tensor_scalar_max(
                        CLA[e][:, 2 * h:2 * h + 2, :], d2[:], 0.0
                    )
                st = stpool.tile([128, 4, 512], F32, tag="st", name="st")
                last_sqrt = nc.scalar.activation(
                    st[:], CLA[e][:], AF.Sqrt, bias=bias12[:]
                )
                KRAW[e] = (st, krpool.tile([128, 4, 512], BF16,
                                           tag=f"kr{e}", name=f"kr{e}"))
            # all exps after all sqrts (ACT table sets differ)
            exp_done = {}
            for e in range(ELEMS):
                st, kraw = KRAW[e]
                exp_inst = nc.scalar.activation(
                    kraw[:], st[:], AF.Exp, scale=-1.0 / EPS,
                )
                tile.add_dep_helper(exp_inst.ins, last_sqrt.ins,
                                    sync=True, reason="act-table-batch")
                exp_done[e] = exp_inst
            # K'A = aw * K via ACT Copy-with-scale; accum -> rowsums
            for e in range(ELEMS):
                kraw = KRAW[e][1]
                for c in range(4):
                    nc.scalar.activation(
                        KA[e][:, c, :], kraw[:, c, :], AF.Copy,
                        scale=aw_v[:, c, e:e + 1],
                        accum_out=rs[:, 4 * c + e:4 * c + e + 1],
                    )
            # K'B = bw * K^T via PE transposes + per-partition-scale evac
            for e in range(ELEMS):
                kraw = KRAW[e][1]
                for cm in range(4):
                    tp = ptp.tile([128, 4, 128], BF16, tag="tp", name="tp")
                    for ci in range(4):
                        nc.tensor.transpose(
                            tp[:, ci, :],
                            kraw[:, ci, cm * 128:(cm + 1) * 128], identB[:],
                        )
                    last_evac[(e, cm)] = nc.vector.tensor_scalar_mul(
                        KB[e][:, cm, :],
                        tp[:].rearrange("p a b -> p (a b)"),
                        bw_v[:, cm, e:e + 1],
                    )
            # first u-update: U~1 = aw / rowsum(K'A)  (v0 = 1 incl. masked)
            rs2 = vpool.tile([128, 16], F32, tag="rs2", name="rs2")
            nc.vector.tensor_scalar_max(rs2[:], rs[:], 1e-30)
            rcp = vpool.tile([128, 16], F32, tag="rcp", name="rcp")
            nc.vector.reciprocal(rcp[:], rs2[:])
            nc.vector.tensor_mul(
                U_all[:, :, :, 0],
                rcp[:].rearrange("p (c e) -> p c e", c=4), aw_v,
            )

        # --- Sinkhorn iterations ----------------------------------------
        with tc.tile_pool(name="psq", bufs=1, space="PSUM") as qpool, \
             tc.tile_pool(name="pt16", bufs=2, space="PSUM") as ptpool, \
             tc.tile_pool(name="dps", bufs=1, space="PSUM") as dpool, \
             tc.tile_pool(name="sb", bufs=2) as sbpool:

            dummy_ps = dpool.tile([1, 512], F32, tag="dps", name="dps")
            # warm-up burst: ~7us of back-to-back matmuls flips the HAM
            # clock gate to 8/8 right before the loop (gated so the
            # PE-idle ACT stretch of setup comes first)
            for i in range(16):
                wm = nc.tensor.matmul(
                    dummy_ps[:], sel[:, 0:1], KA[0][:, 0, :],
                    start=True, stop=True, skip_group_check=True,
                )
                if i == 0:
                    tile.add_dep_helper(wm.ins, last_evac[(1, 3)].ins,
                                        sync=True, reason="warmup-after-setup")

            PSQ = [qpool.tile([128, 256], F32, tag=f"q{q}", name=f"q{q}")
                   for q in range(2)]

            def phase(Kt, stat, out_all):
                """out_all[:,c,e,0] = 1 / (sum_c stat[:,c,e,:]^T @ Kt)"""
                SBQ = []
                for h in range(2):
                    for c in range(4):
                        for e in range(ELEMS):
                            nc.tensor.matmul(
                                PSQ[h][32 * e:32 * e + 32, :],
                                stat[:, c, e, :],
                                Kt[e][:, c, 256 * h:256 * h + 256],
                                start=(c == 0), stop=(c == 3),
                                tile_position=(0, 32 * e),
                            )
                    sbq = sbpool.tile([128, 256], BF16, tag=f"sbq{h}",
                                      name=f"sbq{h}")
                    if h == 0:
                        # ACT (slower) gets the early half: hides under the
                        # second half's matmul streams
                        nc.scalar.copy(sbq[:], PSQ[h][:])
                    else:
                        nc.vector.tensor_copy(sbq[:], PSQ[h][:])
                    SBQ.append(sbq)

                def filler(n, src, after=None):
                    # HAM-warmth fillers anchored on this phase's evacuated
                    # tile (and optionally pinned behind `after`) so the
                    # scheduler cannot hoist them out of the tail's wait
                    # windows / ahead of ready critical ops
                    for _ in range(n):
                        fi = nc.tensor.matmul(
                            dummy_ps[:, 0:128], sel[:, 0:1], src[:, 0:128],
                            start=True, stop=True, skip_group_check=True,
                        )
                        if after is not None:
                            tile.add_dep_helper(fi.ins, after.ins,
                                                sync=True,
                                                reason="filler-after-sel")

                pt16 = ptpool.tile([128, 4, 4], F32, tag="pt16", name="pt16")
                with nc.allow_low_precision("bf16 Sinkhorn potentials"):
                    last_sel = None
                    for c in range(4):
                        last_sel = nc.tensor.matmul(
                            pt16[:, c, :],
                            SBQ[c // 2][:, (c % 2) * 128:(c % 2 + 1) * 128],
                            sel[:], start=True, stop=True,
                        )
                        if c == 1:
                            filler(3, SBQ[0])
                            # early half: unblocks next phase's c0/c1 bursts
                            # (subtile deps) while sel2/sel3 still run
                            nc.vector.reciprocal(out_all[:, 0:2, :, 0],
                                                 pt16[:, 0:2, :])
                    filler(3, SBQ[1], after=last_sel)
                    nc.vector.reciprocal(out_all[:, 2:4, :, 0],
                                         pt16[:, 2:4, :])

            # reference order: 50x(u-update; v-update). u#1 done at setup.
            # DK' = clamp(d2)*K'A muls hide in the loop's DVE slack.
            dk_jobs = [(e, c) for e in range(ELEMS) for c in range(4)]
            nphase = 0
            for _ in range(ITERS - 1):
                for Kt, stat, out_all in ((KA, U_all, V_all),
                                          (KB, V_all, U_all)):
                    phase(Kt, stat, out_all)
                    nphase += 1
                    if nphase - 1 < len(dk_jobs):
                        e, c = dk_jobs[nphase - 1]
                        nc.vector.tensor_mul(
                            DK[e][:, c, :], CLA[e][:, c, :], KA[e][:, c, :]
                        )
            for e, c in dk_jobs[nphase:]:  # leftovers (small ITERS only)
                nc.vector.tensor_mul(
                    DK[e][:, c, :], CLA[e][:, c, :], KA[e][:, c, :]
                )
            phase(KA, U_all, V_all)       # final v-update

            # --- final: ot[e] = (DK'^T U~) . (bw * V~) ------------------
            SBG = []
            for h in range(2):
                for c in range(4):
                    for e in range(ELEMS):
                        nc.tensor.matmul(
                            PSQ[h][32 * e:32 * e + 32, :],
                            U_all[:, c, e, :],
                            DK[e][:, c, 256 * h:256 * h + 256],
                            start=(c == 0), stop=(c == 3),
                            tile_position=(0, 32 * e),
                        )
                sbg = sbpool.tile([128, 256], BF16, tag=f"sbq{h}",
                                  name=f"sbg{h}")
                if h == 0:
                    nc.vector.tensor_copy(sbg[:], PSQ[h][:])
                else:
                    nc.scalar.copy(sbg[:], PSQ[h][:])
                SBG.append(sbg)
            ptg = ptpool.tile([128, 4, 4], F32, tag="pt16", name="ptg")
            for c in range(4):
                nc.tensor.matmul(
                    ptg[:, c, :],
                    SBG[c // 2][:, (c % 2) * 128:(c % 2 + 1) * 128],
                    sel[:], start=True, stop=True,
                )
            t1 = sbpool.tile([128, 4, 4], F32, tag="t1", name="t1")
            nc.vector.tensor_mul(t1[:], ptg[:], V_all[:, :, :, 0])
            t2 = sbpool.tile([128, 4, 4], F32, tag="t2", name="t2")
            nc.vector.tensor_mul(t2[:], t1[:], bw_v)
            r_e = sbpool.tile([128, 4], F32, tag="re", name="re")
            nc.vector.reduce_sum(
                r_e[:], t2[:].rearrange("p c e -> p e c"),
                axis=mybir.AxisListType.X,
            )
            po = ptpool.tile([1, 4], F32, tag="pt16", name="po")
            nc.tensor.matmul(po[:], ones[:], r_e[:], start=True, stop=True)
            nc.scalar.copy(outsb[:], po[:])
            nc.sync.dma_start(out=otp[:], in_=outsb[:])
            # keep the warm-up matmuls live
            scr_sb = vpool.tile([1, 1], F32, tag="scr", name="scr")
            nc.scalar.copy(scr_sb[:], dummy_ps[0:1, 0:1])
            nc.sync.dma_start(out=scrp[:], in_=scr_sb[:])
    nc.compile()
    return nc


_NC_CACHE = {}


def _get_nc():
    if "nc" not in _NC_CACHE:
        _NC_CACHE["nc"] = _build_nc()
    return _NC_CACHE["nc"]


def _host_prep(a_mask, pc_a, b_mask, pc_b):
    """Per-batch-element f32 prep mirroring the reference's masking."""
    f32 = np.float32
    a_pt = (a_mask * pc_a[..., 2]).astype(f32)          # [B,N]
    b_pt = (b_mask * pc_b[..., 2]).astype(f32)          # [B,M]
    va = (a_pt > 0).astype(f32)
    vb = (b_pt > 0).astype(f32)
    aw = (a_pt / a_pt.sum(axis=1, keepdims=True, dtype=f32)).astype(f32)
    bw = (b_pt / b_pt.sum(axis=1, keepdims=True, dtype=f32)).astype(f32)
    xa = pc_a[..., :2].astype(f32)                      # [B,N,2]
    xb = pc_b[..., :2].astype(f32)
    onesN = np.ones((B, N), f32)
    A = np.stack(
        [-2 * xa[..., 0], -2 * xa[..., 1],
         (xa * xa).sum(-1).astype(f32), onesN], axis=1
    ) * va[:, None, :]                                  # [B,4,N]
    Bm = np.stack(
        [xb[..., 0], xb[..., 1], onesN,
         (xb * xb).sum(-1).astype(f32)], axis=1
    ) * vb[:, None, :]                                  # [B,4,M]
    # huber term on host (tiny)
    e = (a_pt.sum(axis=1, dtype=f32) - b_pt.sum(axis=1, dtype=f32)).astype(f32)
    hub = np.where(np.abs(e) <= 1.0, f32(0.5) * e * e, np.abs(e) - f32(0.5))
    chunk = lambda x: x.reshape(B, 4, 128).astype(f32)
    AB = np.concatenate([A.astype(f32), Bm.astype(f32)], axis=2)  # [B,4,1024]
    return AB, chunk(aw), chunk(bw), hub.astype(f32)


def kernel(a_mask, pc_a, b_mask, pc_b, _trace=False):
    AB, aw_pm, bw_pm, hub = _host_prep(
        np.asarray(a_mask), np.asarray(pc_a), np.asarray(b_mask), np.asarray(pc_b)
    )
    eye = np.eye(128, dtype=np.float32)
    in_maps = []
    for core in range(N_CORES):
        sl = slice(core * ELEMS, (core + 1) * ELEMS)
        # [p, (c, e)] layout per weight
        cols = [x[sl].transpose(2, 1, 0).reshape(128, 16)
                for x in (aw_pm, bw_pm)]
        in_maps.append({
            "ABaug": np.ascontiguousarray(AB[sl]),
            "wts": np.ascontiguousarray(np.concatenate(cols, axis=1)),
            "eye": eye,
        })
    nc = _get_nc()
    res = run_bass_kernel_spmd(nc, in_maps, list(range(N_CORES)), trace=_trace)
    ot = np.concatenate([res.results[c]["ot"].reshape(ELEMS) for c in range(N_CORES)])
    out = (ot + hub).astype(np.float32)
    if _trace:
        return out, res
    return out


# revision 37
# speedup vs baseline: 1.1177x; 1.0042x over previous
"""Trainium2 Bass kernel for nn_EnergyMovers (batched Sinkhorn OT loss).

Strategy (pure data parallelism, 4 batch elems per core x 8 cores):
  - Host: build masked augmented point vectors so d2[n,m] = sum_k A[k,n]*B[k,m]
    comes out of a K=4 TensorE matmul already masked (masked rows/cols -> d2=0
    -> K=exp(-sqrt(1e-12)/eps) ~ 1, matching the reference's logK=0 there).
  - Device per elem (A-layout only): d2 -> clamp(DVE) -> sqrt(ACT, fused
    [128,4*512] tile) -> exp(ACT, fused) -> raw K. Then:
      K'A = aw_n * K        via ACT Copy-with-per-partition-scale, whose
                            accum_out also yields rowsum(K'A) for the first
                            u-update (v0 = 1 incl. the reference's masked
                            exp(0)=1 columns) for free,
      K'B = bw_m * K^T      via 128x128 PE transposes of raw K + a
                            per-partition-scale DVE evacuation multiply,
      DK' = clamp(d2)*K'A   as 16 small DVE muls sprinkled one-per-phase
                            into the loop's DVE slack (only needed at the
                            final reduction).
  - Non-log Sinkhorn on reciprocal potentials (u = aw*U~, v = bw*V~):
        s_v = K'A^T @ U~ ; V~ = 1/s_v ; s_u = K'B^T @ V~ ; U~ = 1/s_u
    identical to the reference's log-domain iteration (f32/bf16 exponent
    range suffices).
  - Each phase is QUARTER-PIPELINED on the PE: the matvec output is split
    into four [128,128] PSUM quarters; each quarter's 4 contraction bursts
    (4 batch elems column-tiled at tile_position (0,32e), streaming
    concurrently) complete early, so its evacuation (alternating DVE/ACT,
    distinct tiles to avoid Tile's same-PSUM-reader serialization) and its
    selector matmul overlap the later quarters' streams. sel[32e,e]=1
    picks the 4 result rows -> pt16[128,(c,e)] partition-major, then ONE
    strided DVE reciprocal produces the next stationaries. The M=32
    stationary has the potential chunk in col 0, zeros elsewhere
    (zero-fills unused PSUM rows -> evacuations read no garbage).
  - A ~7us dummy-matmul burst right before the loop flips the HAM clock
    gate to 8/8 so the loop's matmuls run at 2.4 GHz; in-loop PE gaps stay
    well under the ~3.4us MID window so it never re-throttles.
  - Final: ot[e] = (DK'^T U~) . (bw * V~) via one more quarter-pipelined
    matvec + selector matmuls + DVE muls + ones-matmul partition reduction.
  - Host: huber(e) added, results gathered from 8 cores.
"""

import os
from contextlib import ExitStack

import numpy as np

import concourse.bass as bass
import concourse.bacc as bacc
import concourse.mybir as mybir
import concourse.tile as tile
from concourse.bass_utils import run_bass_kernel_spmd

N_CORES = 8
ELEMS = 4  # batch elements per core (B=32 / 8)
B, N, M = 32, 512, 512
EPS = 0.05
ITERS = int(os.environ.get("EM_ITERS", "50"))
F32 = mybir.dt.float32
BF16 = mybir.dt.bfloat16
AF = mybir.ActivationFunctionType
ALU = mybir.AluOpType


def _build_nc():
    nc = bacc.Bacc()
    ABaug = nc.declare_dram_parameter("ABaug", [ELEMS, 4, 2 * N],
                                      mybir.dt.float32r, isOutput=False)
    # wts cols: 0:16 aw[(c,e)], 16:32 bw[(c,e)]
    wtsp = nc.declare_dram_parameter("wts", [128, 32], F32, isOutput=False)
    eyep = nc.declare_dram_parameter("eye", [128, 128], F32, isOutput=False)
    otp = nc.declare_dram_parameter("ot", [1, ELEMS], F32, isOutput=True)
    # keeps the HAM warm-up matmuls live through dead-code elim
    scrp = nc.declare_dram_parameter("scr", [1, 1], F32, isOutput=True)

    with ExitStack() as ctx:
        tc = ctx.enter_context(tile.TileContext(nc))
        kpool = ctx.enter_context(tc.tile_pool(name="kmat", bufs=1))
        vpool = ctx.enter_context(tc.tile_pool(name="vec", bufs=1))

        # --- params / constants -----------------------------------------
        wt_sb = vpool.tile([128, 32], F32, tag="wt", name="wt")
        nc.sync.dma_start(out=wt_sb[:], in_=wtsp[:])
        aw_v = wt_sb[:, 0:16].rearrange("p (c e) -> p c e", c=4)
        bw_v = wt_sb[:, 16:32].rearrange("p (c e) -> p c e", c=4)
        eye_sb = vpool.tile([128, 128], F32, tag="eyef", name="eyef")
        nc.sync.dma_start(out=eye_sb[:], in_=eyep[:])
        identB = vpool.tile([128, 128], BF16, tag="identB", name="identB")
        nc.vector.tensor_copy(identB[:], eye_sb[:])
        ones = vpool.tile([128, 1], F32, tag="ones", name="ones")
        nc.gpsimd.memset(ones[:], 1.0)
        sel = vpool.tile([128, 4], BF16, tag="sel", name="sel")
        nc.gpsimd.memset(sel[:], 0.0)
        for e in range(ELEMS):
            nc.gpsimd.memset(sel[32 * e:32 * e + 1, e:e + 1], 1.0)
        outsb = vpool.tile([1, ELEMS], F32, tag="outsb", name="outsb")
        bias12 = vpool.tile([128, 1], F32, tag="bias12", name="bias12")
        nc.gpsimd.memset(bias12[:], 1e-12)

        # potentials: [128, (c, e, 32)] bf16; col 0 of each 32-block is the
        # live value, cols 1-31 stay zero (zero-pads the M=32 stationary).
        U_all = vpool.tile([128, 4, 4, 32], BF16, tag="U", name="U")
        V_all = vpool.tile([128, 4, 4, 32], BF16, tag="V", name="V")
        nc.gpsimd.memset(U_all[:], 0.0)
        nc.gpsimd.memset(V_all[:], 0.0)

        KA, KB, DK, CLA, AB_SB = {}, {}, {}, {}, {}
        for e in range(ELEMS):
            ab_sb = vpool.tile([4, 2 * N], mybir.dt.float32r,
                               tag=f"ABs{e}", name=f"ABs{e}")
            nc.sync.dma_start(out=ab_sb[:], in_=ABaug[e])
            AB_SB[e] = (ab_sb[:, 0:N], ab_sb[:, N:2 * N])
            KA[e] = kpool.tile([128, 4, 512], BF16, tag=f"KA{e}", name=f"KA{e}")
            KB[e] = kpool.tile([128, 4, 512], BF16, tag=f"KB{e}", name=f"KB{e}")
            DK[e] = kpool.tile([128, 4, 512], BF16, tag=f"DK{e}", name=f"DK{e}")
            CLA[e] = kpool.tile([128, 4, 512], F32, tag=f"cl{e}", name=f"cl{e}")

        rs = vpool.tile([128, 16], F32, tag="rs", name="rs")
        last_evac = {}
        with tc.tile_pool(name="pd2", bufs=3, space="PSUM") as pd2, \
             tc.tile_pool(name="ptp", bufs=2, space="PSUM") as ptp, \
             tc.tile_pool(name="kr", bufs=1) as krpool, \
             tc.tile_pool(name="st", bufs=4) as stpool:
            KRAW = {}
            last_sqrt = None
            # d2 (A layout) -> clamp -> sqrt, fused [128,2,512]/[128,4,512]
            for e in range(ELEMS):
                a_sb, b_sb = AB_SB[e]
                for h in range(2):
                    d2 = pd2.tile([128, 2, 512], F32, tag="d2", name="d2")
                    for c2 in range(2):
                        c = 2 * h + c2
                        nc.tensor.matmul(
                            d2[:, c2, :], a_sb[:, c * 128:(c + 1) * 128],
                            b_sb[:], start=True, stop=True,
                        )
                    nc.vector.tensor_scalar_max(
                        CLA[e][:, 2 * h:2 * h + 2, :], d2[:], 0.0
                    )
                st = stpool.tile([128, 4, 512], F32, tag="st", name="st")
                last_sqrt = nc.scalar.activation(
                    st[:], CLA[e][:], AF.Sqrt, bias=bias12[:]
                )
                KRAW[e] = (st, krpool.tile([128, 4, 512], BF16,
                                           tag=f"kr{e}", name=f"kr{e}"))
            # all exps after all sqrts (ACT table sets differ); per-chunk so
            # accum_out yields rowsum(K) for the first u-update for free
            for e in range(ELEMS):
                st, kraw = KRAW[e]
                for c in range(4):
                    exp_inst = nc.scalar.activation(
                        kraw[:, c, :], st[:, c, :], AF.Exp, scale=-1.0 / EPS,
                        accum_out=rs[:, 4 * c + e:4 * c + e + 1],
                    )
                    tile.add_dep_helper(exp_inst.ins, last_sqrt.ins,
                                        sync=True, reason="act-table-batch")
            # K'A = aw * K on DVE (keeps the serial ACT queue to sqrt+exp)
            for e in range(ELEMS):
                kraw = KRAW[e][1]
                for c in range(4):
                    nc.vector.tensor_scalar_mul(
                        KA[e][:, c, :], kraw[:, c, :], aw_v[:, c, e:e + 1],
                    )
            # K'B = bw * K^T via PE transposes + per-partition-scale evac
            for e in range(ELEMS):
                kraw = KRAW[e][1]
                for cm in range(4):
                    tp = ptp.tile([128, 4, 128], BF16, tag="tp", name="tp")
                    for ci in range(4):
                        nc.tensor.transpose(
                            tp[:, ci, :],
                            kraw[:, ci, cm * 128:(cm + 1) * 128], identB[:],
                        )
                    last_evac[(e, cm)] = nc.vector.tensor_scalar_mul(
                        KB[e][:, cm, :],
                        tp[:].rearrange("p a b -> p (a b)"),
                        bw_v[:, cm, e:e + 1],
                    )
            # first u-update: U~1 = aw/rowsum(K'A) = 1/rowsum(K) since
            # rowsum(K'A) = aw*rowsum(K)  (v0 = 1 incl. masked exp(0)=1)
            rs2 = vpool.tile([128, 16], F32, tag="rs2", name="rs2")
            nc.vector.tensor_scalar_max(rs2[:], rs[:], 1e-30)
            with nc.allow_low_precision("bf16 Sinkhorn potentials"):
                nc.vector.reciprocal(
                    U_all[:, :, :, 0],
                    rs2[:].rearrange("p (c e) -> p c e", c=4),
                )

        # --- Sinkhorn iterations ----------------------------------------
        with tc.tile_pool(name="psq", bufs=1, space="PSUM") as qpool, \
             tc.tile_pool(name="pt16", bufs=2, space="PSUM") as ptpool, \
             tc.tile_pool(name="dps", bufs=1, space="PSUM") as dpool, \
             tc.tile_pool(name="sb", bufs=2) as sbpool:

            dummy_ps = dpool.tile([1, 512], F32, tag="dps", name="dps")
            # warm-up burst: ~7us of back-to-back matmuls flips the HAM
            # clock gate to 8/8 right before the loop (gated so the
            # PE-idle ACT stretch of setup comes first)
            for i in range(16):
                wm = nc.tensor.matmul(
                    dummy_ps[:], sel[:, 0:1], KA[0][:, 0, :],
                    start=True, stop=True, skip_group_check=True,
                )
                if i == 0:
                    tile.add_dep_helper(wm.ins, last_evac[(1, 3)].ins,
                                        sync=True, reason="warmup-after-setup")

            PSQ = [qpool.tile([128, 256], F32, tag=f"q{q}", name=f"q{q}")
                   for q in range(2)]

            def phase(Kt, stat, out_all):
                """out_all[:,c,e,0] = 1 / (sum_c stat[:,c,e,:]^T @ Kt)"""
                SBQ = []
                for h in range(2):
                    for c in range(4):
                        for e in range(ELEMS):
                            nc.tensor.matmul(
                                PSQ[h][32 * e:32 * e + 32, :],
                                stat[:, c, e, :],
                                Kt[e][:, c, 256 * h:256 * h + 256],
                                start=(c == 0), stop=(c == 3),
                                tile_position=(0, 32 * e),
                            )
                    sbq = sbpool.tile([128, 256], BF16, tag=f"sbq{h}",
                                      name=f"sbq{h}")
                    if h == 0:
                        # ACT (slower) gets the early half: hides under the
                        # second half's matmul streams
                        nc.scalar.copy(sbq[:], PSQ[h][:])
                    else:
                        nc.vector.tensor_copy(sbq[:], PSQ[h][:])
                    SBQ.append(sbq)

                def filler(n, src, after=None):
                    # HAM-warmth fillers anchored on this phase's evacuated
                    # tile (and optionally pinned behind `after`) so the
                    # scheduler cannot hoist them out of the tail's wait
                    # windows / ahead of ready critical ops
                    for _ in range(n):
                        fi = nc.tensor.matmul(
                            dummy_ps[:, 0:128], sel[:, 0:1], src[:, 0:128],
                            start=True, stop=True, skip_group_check=True,
                        )
                        if after is not None:
                            tile.add_dep_helper(fi.ins, after.ins,
                                                sync=True,
                                                reason="filler-after-sel")

                pt16 = ptpool.tile([128, 4, 4], F32, tag="pt16", name="pt16")
                with nc.allow_low_precision("bf16 Sinkhorn potentials"):
                    last_sel = None
                    for c in range(4):
                        last_sel = nc.tensor.matmul(
                            pt16[:, c, :],
                            SBQ[c // 2][:, (c % 2) * 128:(c % 2 + 1) * 128],
                            sel[:], start=True, stop=True,
                        )
                        if c == 1:
                            filler(3, SBQ[0])
                            # early half: unblocks next phase's c0/c1 bursts
                            # (subtile deps) while sel2/sel3 still run
                            nc.vector.reciprocal(out_all[:, 0:2, :, 0],
                                                 pt16[:, 0:2, :])
                    filler(3, SBQ[1], after=last_sel)
                    nc.vector.reciprocal(out_all[:, 2:4, :, 0],
                                         pt16[:, 2:4, :])

            # reference order: 50x(u-update; v-update). u#1 done at setup.
            # DK' = clamp(d2)*K'A muls hide in the loop's DVE slack.
            dk_jobs = [(e, c) for e in range(ELEMS) for c in range(4)]
            nphase = 0
            for _ in range(ITERS - 1):
                for Kt, stat, out_all in ((KA, U_all, V_all),
                                          (KB, V_all, U_all)):
                    phase(Kt, stat, out_all)
                    nphase += 1
                    if nphase - 1 < len(dk_jobs):
                        e, c = dk_jobs[nphase - 1]
                        nc.vector.tensor_mul(
                            DK[e][:, c, :], CLA[e][:, c, :], KA[e][:, c, :]
                        )
            for e, c in dk_jobs[nphase:]:  # leftovers (small ITERS only)
                nc.vector.tensor_mul(
                    DK[e][:, c, :], CLA[e][:, c, :], KA[e][:, c, :]
                )
            phase(KA, U_all, V_all)       # final v-update

            # --- final: ot[e] = (DK'^T U~) . (bw * V~) ------------------
            SBG = []
            for h in range(2):
                for c in range(4):
                    for e in range(ELEMS):
                        nc.tensor.matmul(
                            PSQ[h][32 * e:32 * e + 32, :],
                            U_all[:, c, e, :],
                            DK[e][:, c, 256 * h:256 * h + 256],
                            start=(c == 0), stop=(c == 3),
                            tile_position=(0, 32 * e),
                        )
                sbg = sbpool.tile([128, 256], BF16, tag=f"sbq{h}",
                                  name=f"sbg{h}")
                if h == 0:
                    nc.vector.tensor_copy(sbg[:], PSQ[h][:])
                else:
                    nc.scalar.copy(sbg[:], PSQ[h][:])
                SBG.append(sbg)
            ptg = ptpool.tile([128, 4, 4], F32, tag="pt16", name="ptg")
            for c in range(4):
                nc.tensor.matmul(
                    ptg[:, c, :],
                    SBG[c // 2][:, (c % 2) * 128:(c % 2 + 1) * 128],
                    sel[:], start=True, stop=True,
                )
            t1 = sbpool.tile([128, 4, 4], F32, tag="t1", name="t1")
            nc.vector.tensor_mul(t1[:], ptg[:], V_all[:, :, :, 0])
            t2 = sbpool.tile([128, 4, 4], F32, tag="t2", name="t2")
            nc.vector.tensor_mul(t2[:], t1[:], bw_v)
            r_e = sbpool.tile([128, 4], F32, tag="re", name="re")
            nc.vector.reduce_sum(
                r_e[:], t2[:].rearrange("p c e -> p e c"),
                axis=mybir.AxisListType.X,
            )
            po = ptpool.tile([1, 4], F32, tag="pt16", name="po")
            nc.tensor.matmul(po[:], ones[:], r_e[:], start=True, stop=True)
            nc.scalar.copy(outsb[:], po[:])
            nc.sync.dma_start(out=otp[:], in_=outsb[:])
            # keep the warm-up matmuls live
            scr_sb = vpool.tile([1, 1], F32, tag="scr", name="scr")
            nc.scalar.copy(scr_sb[:], dummy_ps[0:1, 0:1])
            nc.sync.dma_start(out=scrp[:], in_=scr_sb[:])
    nc.compile()
    return nc


_NC_CACHE = {}


def _get_nc():
    if "nc" not in _NC_CACHE:
        _NC_CACHE["nc"] = _build_nc()
    return _NC_CACHE["nc"]


def _host_prep(a_mask, pc_a, b_mask, pc_b):
    """Per-batch-element f32 prep mirroring the reference's masking."""
    f32 = np.float32
    a_pt = (a_mask * pc_a[..., 2]).astype(f32)          # [B,N]
    b_pt = (b_mask * pc_b[..., 2]).astype(f32)          # [B,M]
    va = (a_pt > 0).astype(f32)
    vb = (b_pt > 0).astype(f32)
    aw = (a_pt / a_pt.sum(axis=1, keepdims=True, dtype=f32)).astype(f32)
    bw = (b_pt / b_pt.sum(axis=1, keepdims=True, dtype=f32)).astype(f32)
    xa = pc_a[..., :2].astype(f32)                      # [B,N,2]
    xb = pc_b[..., :2].astype(f32)
    onesN = np.ones((B, N), f32)
    A = np.stack(
        [-2 * xa[..., 0], -2 * xa[..., 1],
         (xa * xa).sum(-1).astype(f32), onesN], axis=1
    ) * va[:, None, :]                                  # [B,4,N]
    Bm = np.stack(
        [xb[..., 0], xb[..., 1], onesN,
         (xb * xb).sum(-1).astype(f32)], axis=1
    ) * vb[:, None, :]                                  # [B,4,M]
    # huber term on host (tiny)
    e = (a_pt.sum(axis=1, dtype=f32) - b_pt.sum(axis=1, dtype=f32)).astype(f32)
    hub = np.where(np.abs(e) <= 1.0, f32(0.5) * e * e, np.abs(e) - f32(0.5))
    chunk = lambda x: x.reshape(B, 4, 128).astype(f32)
    AB = np.concatenate([A.astype(f32), Bm.astype(f32)], axis=2)  # [B,4,1024]
    return AB, chunk(aw), chunk(bw), hub.astype(f32)


def kernel(a_mask, pc_a, b_mask, pc_b, _trace=False):
    AB, aw_pm, bw_pm, hub = _host_prep(
        np.asarray(a_mask), np.asarray(pc_a), np.asarray(b_mask), np.asarray(pc_b)
    )
    eye = np.eye(128, dtype=np.float32)
    in_maps = []
    for core in range(N_CORES):
        sl = slice(core * ELEMS, (core + 1) * ELEMS)
        # [p, (c, e)] layout per weight
        cols = [x[sl].transpose(2, 1, 0).reshape(128, 16)
                for x in (aw_pm, bw_pm)]
        in_maps.append({
            "ABaug": np.ascontiguousarray(AB[sl]),
            "wts": np.ascontiguousarray(np.concatenate(cols, axis=1)),
            "eye": eye,
        })
    nc = _get_nc()
    res = run_bass_kernel_spmd(nc, in_maps, list(range(N_CORES)), trace=_trace)
    ot = np.concatenate([res.results[c]["ot"].reshape(ELEMS) for c in range(N_CORES)])
    out = (ot + hub).astype(np.float32)
    if _trace:
        return out, res
    return out
